# revision 31
# baseline (speedup 1.0000x reference)
"""Trainium2 Bass kernel for DinoVisionTransformer Sparse-MoE FC2 (LoRA experts).

Computation (per token t):
    logits = x @ Wg                      -> top-2 softmax-renormalized weights
    out    = x @ W2 + b2 + sum_e cw[t,e] * scale[e] * (x @ A_e) @ B_e

Sharding: data-parallel over the batch dim (8 batch rows -> 8 NeuronCores,
1024 tokens each). All weights replicated.

Per-core kernel (fp16 base path, fp8e4m3 DoubleRow LoRA path, fp32 PSUM):
  All weight scales are folded by 64 (W2*64 fp16, A*64 fp8, Bm*scale*64 fp8)
  so PSUM accumulates 64*(base + delta); the final DVE pass multiplies by
  2^-6 and adds b2.
  Phase A (per pair of 128-deep k-chunks, contraction over H=4096):
    base: x16 stationary, W2 columns fp16 (2x512 per chunk)
    router hi: x16 @ [Wg_hi | Wg_lo] -> ps_l[0:16] (fp16, fp32 accum)
    LoRA: x8 pair stationary, A8 pair moving, fp8 DoubleRow (2 chunks/instr,
      2 cols/cycle) -> ps_h = 64*h
    router lo: xlo8 (= (x - fp16(x))*4096 in fp8) @ wg8 (=Wg*64 fp8)
      DoubleRow -> ps_l[16:24] = 2^18 * correction
  Router (DVE): logits = reduce(ps_l[0:16]) + 2^-18*ps_l[16:24]; top-2 of 8
    via max8; w1 = sigmoid(l1-l2), w2 = 1-w1; dense cw by equality masks.
  hw8 = fp8(ps_h * cw * 2^-6)  (true h*cw scale), PE-transposed in fp8,
  phase B: 4 fp8 DoubleRow matmuls accumulate 64*delta into ps_base.
  Final: out = ps_base * 2^-6 + b2 (DVE scalar_tensor_tensor), DMA out.
"""

import sys

if "/opt/trn_rl_repo" not in sys.path:
    sys.path.insert(0, "/opt/trn_rl_repo")

import numpy as np
import ml_dtypes

import concourse.bass as bass  # noqa: F401  (registers types)
import concourse.mybir as mybir
import concourse.tile as tile
from concourse import bacc
from concourse.bass import ts
from concourse.bass_utils import run_bass_kernel_spmd
from concourse.masks import make_identity

P = 128
KCH = 32          # H / 128 contraction chunks
NPAIR = 16        # DoubleRow k-chunk pairs per tile
TT = 8            # 128-token tiles per core
H = 4096
D = 1024
E = 8
R = 64
ER = E * R        # 512
NW = D + 8 + 8    # 1040 fp16 wcat columns: [W2*64 | Wg_hi | Wg_lo]
NCORES = 8
XLO_S = 4096.0    # host scale on xlo before fp8 quantization
W_S = 64.0        # host scale on W2/A/Bm before quantization
CORR_S = 1.0 / (XLO_S * W_S)   # ps_l[16:24] -> logit units
OUT_S = 1.0 / W_S              # ps_base -> output units

F8 = mybir.dt.float8e4
F16 = mybir.dt.float16
F32 = mybir.dt.float32
DR = mybir.MatmulPerfMode.DoubleRow

_CACHE = {}


def _build_nc():
    nc = bacc.Bacc("TRN2")

    x16_d = nc.dram_tensor("x16", [TT, P, KCH, P], F16, kind="ExternalInput")
    xlo_d = nc.dram_tensor("xlo", [TT, P, KCH, P], F8, kind="ExternalInput")
    wcat_d = nc.dram_tensor("wcat", [P, KCH, NW], F16, kind="ExternalInput")
    a8_d = nc.dram_tensor("a8", [P, KCH, ER], F8, kind="ExternalInput")
    wg8_d = nc.dram_tensor("wg8", [P, KCH, 8], F8, kind="ExternalInput")
    bm_d = nc.dram_tensor("bm", [P, 4, D], F8, kind="ExternalInput")
    b2b_d = nc.dram_tensor("b2b", [P, D], F32, kind="ExternalInput")
    y_d = nc.dram_tensor("y", [TT * P, D], F32, kind="ExternalOutput")

    Sig = mybir.ActivationFunctionType.Sigmoid
    Alu = mybir.AluOpType

    with tile.TileContext(nc) as tc:
        with (
            tc.tile_pool(name="wres", bufs=1) as wres,
            tc.tile_pool(name="xin", bufs=3) as xin,
            tc.tile_pool(name="small", bufs=2) as small,
            tc.tile_pool(name="hbuf", bufs=2) as hbuf,
            tc.tile_pool(name="obuf", bufs=2) as obuf,
            tc.tile_pool(name="ps_base", bufs=2, space="PSUM") as ps_base_pool,
            tc.tile_pool(name="ps_h", bufs=2, space="PSUM") as ps_h_pool,
            tc.tile_pool(name="ps_l", bufs=1, space="PSUM") as ps_l_pool,
            tc.tile_pool(name="ps_t", bufs=1, space="PSUM") as ps_t_pool,
        ):
            # ---- startup DMA. The DMA engines share bandwidth round-robin
            # across outstanding transfers, so the critical-path streams
            # (x16 of tile 0, wcat groups) are issued FIRST from sync (the
            # empirically fast path), while everything needed only from
            # mid-tile-0 onward (xlo, a8 m1+, wg8, bm, b2b) is issued from
            # the vector engine AFTER the x8 cast of tile 0 — a real
            # dependency that keeps those transfers out of the early
            # bandwidth fight. ----
            xts = {}
            late_batches = {0: [], 1: [], 2: []}  # drained after cast(t)

            def issue_x(t0, x16_eng, xlo_eng):
                # x8 is not streamed: it is cast on-device from x16 (saves
                # 4.2 MB of HBM traffic). Cast is emitted in alloc_psums.
                x16_ = xin.tile([P, KCH, P], F16, tag="x16")
                x8_ = xin.tile([P, KCH, P], F8, tag="x8")
                xlo_ = xin.tile([P, KCH, P], F8, tag="xlo")
                if x16_eng is not None:
                    x16_eng.dma_start(x16_[:], x16_d[t0])
                if xlo_eng is not None:
                    xlo_eng.dma_start(xlo_[:], xlo_d[t0])
                xts[t0] = (x16_, x8_, xlo_)
                return x16_, xlo_

            ident = wres.tile([P, P], F16, tag="ident")
            make_identity(nc, ident[:])
            # x16 of tiles 0/1 split in two half-tile DMAs each: a single
            # transfer runs at ~170 GB/s (one HW-DGE queue); splitting gets
            # queue-parallelism on the critical startup stream
            _, xlo0 = issue_x(0, None, None)
            _, xlo1 = issue_x(1, None, None)
            nc.sync.dma_start(xts[0][0][:, 0:KCH // 2, :], x16_d[0, :, 0:KCH // 2, :])
            nc.sync.dma_start(xts[0][0][:, KCH // 2:, :], x16_d[0, :, KCH // 2:, :])
            wcat_sb = []
            a8_sb = []
            for g in range(16):
                t_ = wres.tile([P, 2, NW], F16, tag=f"wcat{g}")
                nc.sync.dma_start(t_[:], wcat_d[:, ts(g, 2), :])
                wcat_sb.append(t_)
                if g == 1:
                    nc.sync.dma_start(
                        xts[1][0][:, 0:KCH // 2, :], x16_d[1, :, 0:KCH // 2, :]
                    )
                    nc.sync.dma_start(
                        xts[1][0][:, KCH // 2:, :], x16_d[1, :, KCH // 2:, :]
                    )
                if g == 5:
                    a_ = wres.tile([P, 8, ER], F8, tag="a80")
                    nc.sync.dma_start(a_[:], a8_d[:, ts(0, 8), :])
                    a8_sb.append(a_)
            wg8_sb = wres.tile([P, KCH, 8], F8, tag="wg8")
            bm_sb = wres.tile([P, 4, D], F8, tag="bm")
            b2b_sb = wres.tile([P, D], F32, tag="b2b")
            for m in range(1, 4):
                a_ = wres.tile([P, 8, ER], F8, tag=f"a8{m}", name=f"a8{m}")
                a8_sb.append(a_)
            x16_2, xlo_2 = issue_x(2, None, None)
            x16_3, xlo_3 = issue_x(3, None, None)
            # deferred issues, in consumption order, drained on gpsimd after
            # each early cast — so their transfers start only once the
            # critical tile-0/1 x16 + wcat stream has landed
            late_batches[0] = [
                (xlo0[:], xlo_d[0]),
                (wg8_sb[:], wg8_d[:]),
                (a8_sb[1][:], a8_d[:, ts(1, 8), :]),
                (a8_sb[2][:], a8_d[:, ts(2, 8), :]),
                (a8_sb[3][:], a8_d[:, ts(3, 8), :]),
            ]
            late_batches[1] = [
                (xlo1[:], xlo_d[1]),
                (bm_sb[:], bm_d[:]),
                (b2b_sb[:], b2b_d[:]),
                (x16_2[:], x16_d[2]),
                (xlo_2[:], xlo_d[2]),
            ]
            late_batches[2] = [
                (x16_3[:], x16_d[3]),
                (xlo_3[:], xlo_d[3]),
            ]
            # drain the deferred batches on gpsimd, each gated behind a tiny
            # copy that reads the corresponding x16 tile — so these
            # transfers start only after the critical early streams landed.
            # Batch 2 needs no gate: its destinations reuse tile-0 xin
            # buffers, so the WAR dependency throttles them naturally.
            for gi in (0, 1):
                gate_ = small.tile([P, 8], F16, tag="gate", name=f"gate{gi}")
                nc.gpsimd.tensor_copy(gate_[:], xts[gi][0][:, 0, 0:8])
                for dst, src in late_batches.pop(gi):
                    nc.gpsimd.dma_start(dst, src)
            for dst, src in late_batches.pop(2):
                nc.gpsimd.dma_start(dst, src)

            def wc(k, lo, hi):
                return wcat_sb[k // 2][:, k % 2, lo:hi]

            def a8p(j):
                return a8_sb[j // 4][:, (j % 4) * 2:(j % 4) * 2 + 2, :]

            # shared logits psum bank: tile t uses half (t % 2).
            # cols [0:16] = x16 @ [Wg_hi | Wg_lo]; cols [16:24] = 2^18 x the
            # xlo correction (fp8 DoubleRow; rescaled on the DVE afterwards)
            ps_l_shared = ps_l_pool.tile([P, 64], F32, tag="l")

            pend = {}   # t -> (ps_base, ps_h, hwT or None)

            def emit_A_pair(t, j, late8=False, warm_only=False):
                """Phase-A matmuls for k-chunk pair j (chunks 2j, 2j+1).

                late8: bunch the fp8 LoRA + xlo-correction DoubleRow matmuls
                into the second half of the pair loop (two per slot) so the
                fp8 x streams can be issued after the first wcat groups."""
                x16_sb, x8_sb, xlo_sb = xts[t]
                ps_base, ps_h, _ = pend[t]
                ps_l = ps_l_shared[:, (t % 2) * 32:(t % 2) * 32 + 32]

                def lora(jj):
                    nc.tensor.matmul(
                        ps_h[:, :], x8_sb[:, ts(jj, 2), :], a8p(jj),
                        start=False, stop=(jj == NPAIR - 1),
                        perf_mode=DR, skip_group_check=True,
                    )

                def xcorr(jj):
                    nc.tensor.matmul(
                        ps_l[:, 16:24], xlo_sb[:, ts(jj, 2), :],
                        wg8_sb[:, ts(jj, 2), :],
                        start=False, stop=(jj == NPAIR - 1),
                        perf_mode=DR, skip_group_check=True,
                    )

                for k in (2 * j, 2 * j + 1):
                    st = k == 0
                    # order: tiny-N matmuls sit between 512-col streams so
                    # their self-loading weight fetches hide under the streams
                    nc.tensor.matmul(
                        ps_base[:, 0:512], x16_sb[:, k, :], wc(k, 0, 512),
                        start=st, stop=False, skip_group_check=True,
                    )
                    if not warm_only:
                        nc.tensor.matmul(
                            ps_l[:, 0:16], x16_sb[:, k, :], wc(k, D, NW),
                            start=False, stop=False, skip_group_check=True,
                        )
                    nc.tensor.matmul(
                        ps_base[:, 512:1024], x16_sb[:, k, :], wc(k, 512, 1024),
                        start=st, stop=False, skip_group_check=True,
                    )
                    if warm_only:
                        continue
                    if k % 2 == 1:
                        if late8:
                            if j >= NPAIR // 2:
                                for jj in (j - NPAIR // 2, j):
                                    lora(jj)
                                    xcorr(jj)
                        else:
                            lora(j)
                            xcorr(j)

            def emit_router_dve(t):
                """Router math + h-weighting (DVE/ACT only); returns hw8."""
                ps_base, ps_h, _ = pend[t]
                ps_l = ps_l_shared[:, (t % 2) * 32:(t % 2) * 32 + 32]
                logits = small.tile([P, 8], F32, tag="logits")
                nc.vector.tensor_reduce(
                    logits[:],
                    ps_l[:, 0:16].rearrange("p (s j) -> p j s", s=2),
                    axis=mybir.AxisListType.X,
                    op=Alu.add,
                )
                nc.vector.scalar_tensor_tensor(
                    logits[:], ps_l[:, 16:24], CORR_S, logits[:],
                    op0=Alu.mult, op1=Alu.add,
                )
                m8 = small.tile([P, 8], F32, tag="m8")
                nc.vector.max(m8[:], logits[:])
                g_ = small.tile([P, 1], F32, tag="gap")
                nc.vector.tensor_sub(g_[:], m8[:, 0:1], m8[:, 1:2])
                w1 = small.tile([P, 1], F32, tag="w1")
                nc.scalar.activation(w1[:], g_[:], Sig)
                w2 = small.tile([P, 1], F32, tag="w2")
                nc.scalar.activation(w2[:], g_[:], Sig, scale=-1.0)
                cw = small.tile([P, 8], F32, tag="cw")
                cwb = small.tile([P, 8], F32, tag="cwb")
                nc.vector.scalar_tensor_tensor(
                    cw[:], logits[:], m8[:, 0:1], w1[:, 0:1].to_broadcast([P, 8]),
                    op0=Alu.is_equal, op1=Alu.mult,
                )
                nc.vector.scalar_tensor_tensor(
                    cwb[:], logits[:], m8[:, 1:2], w2[:, 0:1].to_broadcast([P, 8]),
                    op0=Alu.is_equal, op1=Alu.mult,
                )
                nc.vector.tensor_add(cw[:], cw[:], cwb[:])
                hw = hbuf.tile([P, ER], F16, tag="hw")
                # hw = (64*h) * 2^-6 * cw -> true h*cw scale; fp16 here so the
                # PE transpose is legal, cast to fp8 on the psum->sbuf copy
                nc.vector.scalar_tensor_tensor(
                    hw.rearrange("p (e r) -> p e r", e=E),
                    ps_h.rearrange("p (e r) -> p e r", e=E),
                    OUT_S,
                    cw[:, :, None].to_broadcast([P, E, R]),
                    op0=Alu.mult, op1=Alu.mult,
                )
                return hw

            def emit_router_pe(t, hw):
                """PE transposes of weighted h + copy back; fills pend[t] hwT."""
                ps_base, ps_h, _ = pend[t]
                ps_t = ps_t_pool.tile([P, ER], F16, tag="t")
                for j in range(4):
                    nc.tensor.transpose(
                        ps_t[:, ts(j, P)], hw[:, ts(j, P)], ident[:]
                    )
                hwT = hbuf.tile([P, 4, P], F8, tag="hwT")
                nc.vector.tensor_copy(hwT.rearrange("p a b -> p (a b)"), ps_t[:])
                pend[t] = (ps_base, ps_h, hwT)

            def emit_router(t):
                emit_router_pe(t, emit_router_dve(t))

            def emit_B_and_out(t):
                """LoRA phase B (fp8 DoubleRow) into base psum, bias, store."""
                ps_base, _, hwT = pend.pop(t)
                for j in range(2):
                    nc.tensor.matmul(
                        ps_base[:, 0:512], hwT[:, ts(j, 2), :],
                        bm_sb[:, ts(j, 2), 0:512],
                        start=False, stop=False,
                        perf_mode=DR, skip_group_check=True,
                    )
                    nc.tensor.matmul(
                        ps_base[:, 512:1024], hwT[:, ts(j, 2), :],
                        bm_sb[:, ts(j, 2), 512:1024],
                        start=False, stop=(j == 1),
                        perf_mode=DR, skip_group_check=True,
                    )
                out_sb = obuf.tile([P, D], F32, tag="out")
                nc.vector.scalar_tensor_tensor(
                    out_sb[:], ps_base[:], OUT_S, b2b_sb[:],
                    op0=Alu.mult, op1=Alu.add,
                )
                nc.scalar.dma_start(y_d[ts(t, P), :], out_sb[:])

            def alloc_psums(t):
                pend[t] = (
                    ps_base_pool.tile([P, D], F32, tag="base", name=f"base{t}"),
                    ps_h_pool.tile([P, ER], F32, tag="h", name=f"h{t}"),
                    None,
                )
                # The shared logits bank must never see start=True (a bank-wide
                # has_written clear would wipe the other tile's half). Instead
                # zero this tile's half; start=False matmuls then accumulate
                # onto 0 (bits set) or overwrite with v (bits clear) — both ok.
                nc.vector.memset(
                    ps_l_shared[:, (t % 2) * 32:(t % 2) * 32 + 32], 0.0
                )
                # ps_h takes only start=False matmuls (DoubleRow), zero it too
                ps_h = pend[t][1]
                nc.vector.memset(ps_h[:], 0.0)
                # on-device x8 cast for this tile (DVE, ~3us, runs in DVE
                # slack well before the tile's first LoRA matmul)
                x16_sb, x8_sb, _ = xts[t]
                nc.vector.tensor_copy(
                    x8_sb.rearrange("p a b -> p (a b)"),
                    x16_sb.rearrange("p a b -> p (a b)"),
                )

            # ---- startup: interleave phase A of tiles 0 and 1 so the PE has
            # two tiles of work while wcat groups stream in ----
            D_OFF = 4
            alloc_psums(0)
            alloc_psums(1)

            for g in range(NPAIR + D_OFF):
                if g < NPAIR:
                    emit_A_pair(0, g, late8=True)
                if g == NPAIR:
                    emit_router(0)
                gg = g - D_OFF
                if 0 <= gg < NPAIR:
                    emit_A_pair(1, gg, late8=True)
                if gg == NPAIR - 4:
                    emit_B_and_out(0)
            hw_pend = {1: emit_router_dve(1)}

            # ---- steady state ----
            for t in range(2, TT - 1):
                if t >= 4:
                    issue_x(t, nc.gpsimd, nc.scalar)
                alloc_psums(t)
                for j in range(NPAIR):
                    emit_A_pair(t, j)
                    if j == 0 and (t - 1) in hw_pend:
                        # previous tile's transposes here: its DVE router
                        # chain is long done, so the PE never stalls on it
                        emit_router_pe(t - 1, hw_pend.pop(t - 1))
                    if j == 8:
                        # previous tile's phase B mid-A so its psum/base slot
                        # frees well before tile t+1 needs it
                        emit_B_and_out(t - 1)
                hw_pend[t] = emit_router_dve(t)

            # ---- last tile: router columns (L, h) stream first so the DVE
            # router chain overlaps the base-column streams; transposes are
            # injected mid-loop -> phase B follows the final matmul directly
            t = TT - 1
            issue_x(t, nc.gpsimd, nc.scalar)
            alloc_psums(t)
            x16_sb, x8_sb, xlo_sb = xts[t]
            ps_base, ps_h, _ = pend[t]
            ps_l = ps_l_shared[:, (t % 2) * 32:(t % 2) * 32 + 32]
            for j in range(NPAIR):
                for k in (2 * j, 2 * j + 1):
                    nc.tensor.matmul(
                        ps_l[:, 0:16], x16_sb[:, k, :], wc(k, D, NW),
                        start=False, stop=False, skip_group_check=True,
                    )
                if j == 2 and (t - 1) in hw_pend:
                    emit_router_pe(t - 1, hw_pend.pop(t - 1))
                if j >= NPAIR // 2:
                    # LoRA bunched late so the on-device x8 cast (DVE) for
                    # this tile completes before its first consumer
                    for jj in (j - NPAIR // 2, j):
                        nc.tensor.matmul(
                            ps_h[:, :], x8_sb[:, ts(jj, 2), :], a8p(jj),
                            start=False, stop=(jj == NPAIR - 1),
                            perf_mode=DR, skip_group_check=True,
                        )
                nc.tensor.matmul(
                    ps_l[:, 16:24], xlo_sb[:, ts(j, 2), :],
                    wg8_sb[:, ts(j, 2), :],
                    start=False, stop=(j == NPAIR - 1),
                    perf_mode=DR, skip_group_check=True,
                )
            hw_last = emit_router_dve(t)
            for k in range(KCH):
                st = k == 0
                nc.tensor.matmul(
                    ps_base[:, 0:512], x16_sb[:, k, :], wc(k, 0, 512),
                    start=st, stop=False, skip_group_check=True,
                )
                nc.tensor.matmul(
                    ps_base[:, 512:1024], x16_sb[:, k, :], wc(k, 512, 1024),
                    start=st, stop=(k == KCH - 1), skip_group_check=True,
                )
                if k == 4:
                    emit_B_and_out(t - 1)
                if k == 10:
                    emit_router_pe(t, hw_last)
            # drain: last tile's phase B with split evac so the first half's
            # bias-add + store overlap the second half's matmuls
            ps_base, _, hwT = pend.pop(TT - 1)
            for j in range(2):
                nc.tensor.matmul(
                    ps_base[:, 0:512], hwT[:, ts(j, 2), :],
                    bm_sb[:, ts(j, 2), 0:512],
                    start=False, stop=(j == 1),
                    perf_mode=DR, skip_group_check=True,
                )
            out_sb = obuf.tile([P, D], F32, tag="out")
            nc.vector.scalar_tensor_tensor(
                out_sb[:, 0:512], ps_base[:, 0:512], OUT_S, b2b_sb[:, 0:512],
                op0=Alu.mult, op1=Alu.add,
            )
            nc.scalar.dma_start(
                y_d[ts(TT - 1, P), 0:512], out_sb[:, 0:512]
            )
            for j in range(2):
                nc.tensor.matmul(
                    ps_base[:, 512:1024], hwT[:, ts(j, 2), :],
                    bm_sb[:, ts(j, 2), 512:1024],
                    start=False, stop=(j == 1),
                    perf_mode=DR, skip_group_check=True,
                )
            nc.vector.scalar_tensor_tensor(
                out_sb[:, 512:1024], ps_base[:, 512:1024], OUT_S,
                b2b_sb[:, 512:1024],
                op0=Alu.mult, op1=Alu.add,
            )
            nc.scalar.dma_start(
                y_d[ts(TT - 1, P), 512:1024], out_sb[:, 512:1024]
            )

    nc.finalize()
    return nc


def _prep_shared(Wg, W2, b2, A, Bm, scale):
    """Host-side weight layout prep (replicated across cores)."""
    f16, f32 = np.float16, np.float32
    f8 = ml_dtypes.float8_e4m3

    def chunked(a):
        # [H, N] -> [P, KCH, N]
        return np.ascontiguousarray(
            a.reshape(KCH, P, -1).transpose(1, 0, 2)
        )

    # wcat = [W2*64 | Wg_hi | Wg_lo] fp16
    wg_hi = Wg.astype(f16)
    wg_lo = (Wg.astype(f32) - wg_hi.astype(f32)).astype(f16)
    wcat = np.empty((H, NW), dtype=f16)
    wcat[:, 0:D] = (W2.astype(f32) * W_S).astype(f16)
    wcat[:, D:D + 8] = wg_hi
    wcat[:, D + 8:] = wg_lo
    wcat = chunked(wcat)

    a_flat = np.ascontiguousarray(A.transpose(1, 0, 2)).reshape(H, ER)
    a8 = chunked((a_flat.astype(f32) * W_S).astype(f8))
    wg8 = chunked((Wg.astype(f32) * W_S).astype(f8))

    # Bm with scale and 64x folded, [(e r), d] -> [128, 4, D] fp8
    bms = (Bm.astype(f32) * scale.astype(f32)[:, None, None]).reshape(ER, D)
    bms = np.ascontiguousarray(
        (bms * W_S).reshape(4, P, D).transpose(1, 0, 2)
    ).astype(f8)

    b2b = np.ascontiguousarray(
        np.broadcast_to(b2.astype(f32)[None, :], (P, D))
    )
    return wcat, a8, wg8, bms, b2b


def _prep_x_core(x_c):
    """Per-core x prep: fp16 hi + scaled-fp8 lo; [t, p, k, ti] layout.
    (x8 is derived on-device from x16.)"""
    f32 = np.float32
    f8 = ml_dtypes.float8_e4m3
    x16 = x_c.astype(np.float16)                            # [1024, 4096]
    xlo = ((x_c.astype(f32) - x16.astype(f32)) * XLO_S).astype(f8)

    def lay(a):
        return np.ascontiguousarray(
            a.reshape(TT, P, KCH, P).transpose(0, 3, 2, 1)
        )
    return lay(x16), lay(xlo)


def kernel(x, Wg, W2, b2, A, Bm, scale):
    x = np.asarray(x, dtype=np.float32)
    Wg = np.asarray(Wg, dtype=np.float32)
    W2 = np.asarray(W2, dtype=np.float32)
    b2 = np.asarray(b2, dtype=np.float32)
    A = np.asarray(A, dtype=np.float32)
    Bm = np.asarray(Bm, dtype=np.float32)
    scale = np.asarray(scale, dtype=np.float32)

    if "nc" not in _CACHE:
        _CACHE["nc"] = _build_nc()
    nc = _CACHE["nc"]

    wcat, a8, wg8, bms, b2b = _prep_shared(Wg, W2, b2, A, Bm, scale)
    in_maps = []
    for c in range(NCORES):
        x16, xlo = _prep_x_core(x[c])
        in_maps.append(
            {"x16": x16, "xlo": xlo, "wcat": wcat, "a8": a8,
             "wg8": wg8, "bm": bms, "b2b": b2b}
        )

    res = run_bass_kernel_spmd(nc, in_maps, core_ids=list(range(NCORES)))
    out = np.stack([res.results[c]["y"] for c in range(NCORES)], axis=0)
    return out.astype(np.float32)


# revision 35
# speedup vs baseline: 1.0229x; 1.0229x over previous
"""Trainium2 Bass kernel for DinoVisionTransformer Sparse-MoE FC2 (LoRA experts).

Computation (per token t):
    logits = x @ Wg                      -> top-2 softmax-renormalized weights
    out    = x @ W2 + b2 + sum_e cw[t,e] * scale[e] * (x @ A_e) @ B_e

Sharding: data-parallel over the batch dim (8 batch rows -> 8 NeuronCores,
1024 tokens each). All weights replicated.

Per-core kernel (fp16 base path, fp8e4m3 DoubleRow LoRA path, fp32 PSUM):
  All weight scales are folded by 64 (W2*64 fp16, A*64 fp8, Bm*scale*64 fp8)
  so PSUM accumulates 64*(base + delta); the final DVE pass multiplies by
  2^-6 and adds b2.
  Phase A (per pair of 128-deep k-chunks, contraction over H=4096):
    base: x16 stationary, W2 columns fp16 (2x512 per chunk)
    router hi: x16 @ [Wg_hi | Wg_lo] -> ps_l[0:16] (fp16, fp32 accum)
    LoRA: x8 pair stationary, A8 pair moving, fp8 DoubleRow (2 chunks/instr,
      2 cols/cycle) -> ps_h = 64*h
    router lo: xlo8 (= (x - fp16(x))*4096 in fp8) @ wg8 (=Wg*64 fp8)
      DoubleRow -> ps_l[16:24] = 2^18 * correction
  Router (DVE): logits = reduce(ps_l[0:16]) + 2^-18*ps_l[16:24]; top-2 of 8
    via max8; w1 = sigmoid(l1-l2), w2 = 1-w1; dense cw by equality masks.
  hw8 = fp8(ps_h * cw * 2^-6)  (true h*cw scale), PE-transposed in fp8,
  phase B: 4 fp8 DoubleRow matmuls accumulate 64*delta into ps_base.
  Final: out = ps_base * 2^-6 + b2 (DVE scalar_tensor_tensor), DMA out.
"""

import sys

if "/opt/trn_rl_repo" not in sys.path:
    sys.path.insert(0, "/opt/trn_rl_repo")

import numpy as np
import ml_dtypes

import concourse.bass as bass  # noqa: F401  (registers types)
import concourse.mybir as mybir
import concourse.tile as tile
from concourse import bacc
from concourse.bass import ts
from concourse.bass_utils import run_bass_kernel_spmd
from concourse.masks import make_identity

P = 128
KCH = 32          # H / 128 contraction chunks
NPAIR = 16        # DoubleRow k-chunk pairs per tile
TT = 8            # 128-token tiles per core
H = 4096
D = 1024
E = 8
R = 64
ER = E * R        # 512
NW = D + 8 + 8    # 1040 fp16 wcat columns: [W2*64 | Wg_hi | Wg_lo]
NCORES = 8
XLO_S = 4096.0    # host scale on xlo before fp8 quantization
W_S = 64.0        # host scale on W2/A/Bm before quantization
CORR_S = 1.0 / (XLO_S * W_S)   # ps_l[16:24] -> logit units
OUT_S = 1.0 / W_S              # ps_base -> output units

F8 = mybir.dt.float8e4
F16 = mybir.dt.float16
F32 = mybir.dt.float32
DR = mybir.MatmulPerfMode.DoubleRow

_CACHE = {}


def _build_nc():
    nc = bacc.Bacc("TRN2")

    x16_d = nc.dram_tensor("x16", [TT, P, KCH, P], F16, kind="ExternalInput")
    xlo_d = nc.dram_tensor("xlo", [TT, P, KCH, P], F8, kind="ExternalInput")
    wcat_d = nc.dram_tensor("wcat", [P, KCH, NW], F16, kind="ExternalInput")
    a8_d = nc.dram_tensor("a8", [P, KCH, ER], F8, kind="ExternalInput")
    wg8_d = nc.dram_tensor("wg8", [P, KCH, 8], F8, kind="ExternalInput")
    bm_d = nc.dram_tensor("bm", [P, 4, D], F8, kind="ExternalInput")
    b2b_d = nc.dram_tensor("b2b", [P, D], F32, kind="ExternalInput")
    y_d = nc.dram_tensor("y", [TT * P, D], F32, kind="ExternalOutput")

    Sig = mybir.ActivationFunctionType.Sigmoid
    Alu = mybir.AluOpType

    with tile.TileContext(nc) as tc:
        with (
            tc.tile_pool(name="wres", bufs=1) as wres,
            tc.tile_pool(name="xin", bufs=3) as xin,
            tc.tile_pool(name="small", bufs=2) as small,
            tc.tile_pool(name="hbuf", bufs=2) as hbuf,
            tc.tile_pool(name="obuf", bufs=2) as obuf,
            tc.tile_pool(name="ps_base", bufs=2, space="PSUM") as ps_base_pool,
            tc.tile_pool(name="ps_h", bufs=2, space="PSUM") as ps_h_pool,
            tc.tile_pool(name="ps_l", bufs=1, space="PSUM") as ps_l_pool,
            tc.tile_pool(name="ps_t", bufs=1, space="PSUM") as ps_t_pool,
        ):
            # ---- startup DMA. The DMA engines share bandwidth round-robin
            # across outstanding transfers, so the critical-path streams
            # (x16 of tile 0, wcat groups) are issued FIRST from sync (the
            # empirically fast path), while everything needed only from
            # mid-tile-0 onward (xlo, a8 m1+, wg8, bm, b2b) is issued from
            # the vector engine AFTER the x8 cast of tile 0 — a real
            # dependency that keeps those transfers out of the early
            # bandwidth fight. ----
            xts = {}
            late_batches = {0: [], 1: [], 2: []}  # drained after cast(t)

            def issue_x(t0, x16_eng, xlo_eng):
                # x8 is not streamed: it is cast on-device from x16 (saves
                # 4.2 MB of HBM traffic). Cast is emitted in alloc_psums.
                x16_ = xin.tile([P, KCH, P], F16, tag="x16")
                x8_ = xin.tile([P, KCH, P], F8, tag="x8")
                xlo_ = xin.tile([P, KCH, P], F8, tag="xlo")
                if x16_eng is not None:
                    x16_eng.dma_start(x16_[:], x16_d[t0])
                if xlo_eng is not None:
                    xlo_eng.dma_start(xlo_[:], xlo_d[t0])
                xts[t0] = (x16_, x8_, xlo_)
                return x16_, xlo_

            ident = wres.tile([P, P], F16, tag="ident")
            make_identity(nc, ident[:])
            # One DMA transfer runs at only ~170 GB/s (single HW-DGE queue
            # set); the critical startup streams are split across ENGINES
            # (separate queue pools) for transfer parallelism without
            # serializing any one engine's issue stream.
            _, xlo0 = issue_x(0, None, None)
            _, xlo1 = issue_x(1, None, None)
            HK = KCH // 2
            nc.sync.dma_start(xts[0][0][:, 0:HK, :], x16_d[0, :, 0:HK, :])
            nc.scalar.dma_start(xts[0][0][:, HK:, :], x16_d[0, :, HK:, :])
            wcat_sb = []
            a8_sb = []
            wcat0_ = wres.tile([P, 4, NW], F16, tag="wcat0", name="wcat0")
            wcat_sb.append(wcat0_)
            nc.sync.dma_start(wcat0_[:, 0:2, :], wcat_d[:, 0:2, :])
            nc.scalar.dma_start(wcat0_[:, 2:4, :], wcat_d[:, 2:4, :])
            nc.gpsimd.dma_start(xts[1][0][:, 0:HK, :], x16_d[1, :, 0:HK, :])
            nc.sync.dma_start(xts[1][0][:, HK:, :], x16_d[1, :, HK:, :])
            for g in range(1, 8):
                t_ = wres.tile([P, 4, NW], F16, tag=f"wcat{g}")
                nc.sync.dma_start(t_[:], wcat_d[:, ts(g, 4), :])
                wcat_sb.append(t_)
                if g == 3:
                    a_ = wres.tile([P, 8, ER], F8, tag="a80")
                    nc.sync.dma_start(a_[:], a8_d[:, ts(0, 8), :])
                    a8_sb.append(a_)
            wg8_sb = wres.tile([P, KCH, 8], F8, tag="wg8")
            bm_sb = wres.tile([P, 4, D], F8, tag="bm")
            b2b_sb = wres.tile([P, D], F32, tag="b2b")
            for m in range(1, 4):
                a_ = wres.tile([P, 8, ER], F8, tag=f"a8{m}", name=f"a8{m}")
                a8_sb.append(a_)
            x16_2, xlo_2 = issue_x(2, None, None)
            x16_3, xlo_3 = issue_x(3, None, None)
            # deferred issues, in consumption order, drained on gpsimd after
            # each early cast — so their transfers start only once the
            # critical tile-0/1 x16 + wcat stream has landed
            late_batches[0] = [
                (xlo0[:], xlo_d[0]),
                (wg8_sb[:], wg8_d[:]),
                (a8_sb[1][:], a8_d[:, ts(1, 8), :]),
                (a8_sb[2][:], a8_d[:, ts(2, 8), :]),
                (a8_sb[3][:], a8_d[:, ts(3, 8), :]),
            ]
            late_batches[1] = [
                (xlo1[:], xlo_d[1]),
                (bm_sb[:], bm_d[:]),
                (b2b_sb[:], b2b_d[:]),
                (x16_2[:], x16_d[2]),
                (xlo_2[:], xlo_d[2]),
            ]
            late_batches[2] = [
                (x16_3[:], x16_d[3]),
                (xlo_3[:], xlo_d[3]),
            ]
            # drain the deferred batches on gpsimd, each gated behind a tiny
            # copy that reads the corresponding x16 tile — so these
            # transfers start only after the critical early streams landed.
            # Batch 2 needs no gate: its destinations reuse tile-0 xin
            # buffers, so the WAR dependency throttles them naturally.
            for gi in (0, 1):
                gate_ = small.tile([P, 8], F16, tag="gate", name=f"gate{gi}")
                nc.gpsimd.tensor_copy(gate_[:], xts[gi][0][:, 0, 0:8])
                for dst, src in late_batches.pop(gi):
                    nc.gpsimd.dma_start(dst, src)
            for dst, src in late_batches.pop(2):
                nc.gpsimd.dma_start(dst, src)

            def wc(k, lo, hi):
                return wcat_sb[k // 4][:, k % 4, lo:hi]

            def a8p(j):
                return a8_sb[j // 4][:, (j % 4) * 2:(j % 4) * 2 + 2, :]

            # shared logits psum bank: tile t uses half (t % 2).
            # cols [0:16] = x16 @ [Wg_hi | Wg_lo]; cols [16:24] = 2^18 x the
            # xlo correction (fp8 DoubleRow; rescaled on the DVE afterwards)
            ps_l_shared = ps_l_pool.tile([P, 64], F32, tag="l")

            pend = {}   # t -> (ps_base, ps_h, hwT or None)

            def emit_A_pair(t, j, late8=False, warm_only=False):
                """Phase-A matmuls for k-chunk pair j (chunks 2j, 2j+1).

                late8: bunch the fp8 LoRA + xlo-correction DoubleRow matmuls
                into the second half of the pair loop (two per slot) so the
                fp8 x streams can be issued after the first wcat groups."""
                x16_sb, x8_sb, xlo_sb = xts[t]
                ps_base, ps_h, _ = pend[t]
                ps_l = ps_l_shared[:, (t % 2) * 32:(t % 2) * 32 + 32]

                def lora(jj):
                    nc.tensor.matmul(
                        ps_h[:, :], x8_sb[:, ts(jj, 2), :], a8p(jj),
                        start=False, stop=(jj == NPAIR - 1),
                        perf_mode=DR, skip_group_check=True,
                    )

                def xcorr(jj):
                    nc.tensor.matmul(
                        ps_l[:, 16:24], xlo_sb[:, ts(jj, 2), :],
                        wg8_sb[:, ts(jj, 2), :],
                        start=False, stop=(jj == NPAIR - 1),
                        perf_mode=DR, skip_group_check=True,
                    )

                for k in (2 * j, 2 * j + 1):
                    st = k == 0
                    # order: tiny-N matmuls sit between 512-col streams so
                    # their self-loading weight fetches hide under the streams
                    nc.tensor.matmul(
                        ps_base[:, 0:512], x16_sb[:, k, :], wc(k, 0, 512),
                        start=st, stop=False, skip_group_check=True,
                    )
                    if not warm_only:
                        nc.tensor.matmul(
                            ps_l[:, 0:16], x16_sb[:, k, :], wc(k, D, NW),
                            start=False, stop=False, skip_group_check=True,
                        )
                    nc.tensor.matmul(
                        ps_base[:, 512:1024], x16_sb[:, k, :], wc(k, 512, 1024),
                        start=st, stop=False, skip_group_check=True,
                    )
                    if warm_only:
                        continue
                    if k % 2 == 1:
                        if late8:
                            if j >= NPAIR // 2:
                                for jj in (j - NPAIR // 2, j):
                                    lora(jj)
                                    xcorr(jj)
                        else:
                            lora(j)
                            xcorr(j)

            def emit_router_dve(t):
                """Router math + h-weighting (DVE/ACT only); returns hw8."""
                ps_base, ps_h, _ = pend[t]
                ps_l = ps_l_shared[:, (t % 2) * 32:(t % 2) * 32 + 32]
                logits = small.tile([P, 8], F32, tag="logits")
                nc.vector.tensor_reduce(
                    logits[:],
                    ps_l[:, 0:16].rearrange("p (s j) -> p j s", s=2),
                    axis=mybir.AxisListType.X,
                    op=Alu.add,
                )
                nc.vector.scalar_tensor_tensor(
                    logits[:], ps_l[:, 16:24], CORR_S, logits[:],
                    op0=Alu.mult, op1=Alu.add,
                )
                m8 = small.tile([P, 8], F32, tag="m8")
                nc.vector.max(m8[:], logits[:])
                g_ = small.tile([P, 1], F32, tag="gap")
                nc.vector.tensor_sub(g_[:], m8[:, 0:1], m8[:, 1:2])
                w1 = small.tile([P, 1], F32, tag="w1")
                nc.scalar.activation(w1[:], g_[:], Sig)
                w2 = small.tile([P, 1], F32, tag="w2")
                nc.scalar.activation(w2[:], g_[:], Sig, scale=-1.0)
                cw = small.tile([P, 8], F32, tag="cw")
                cwb = small.tile([P, 8], F32, tag="cwb")
                nc.vector.scalar_tensor_tensor(
                    cw[:], logits[:], m8[:, 0:1], w1[:, 0:1].to_broadcast([P, 8]),
                    op0=Alu.is_equal, op1=Alu.mult,
                )
                nc.vector.scalar_tensor_tensor(
                    cwb[:], logits[:], m8[:, 1:2], w2[:, 0:1].to_broadcast([P, 8]),
                    op0=Alu.is_equal, op1=Alu.mult,
                )
                nc.vector.tensor_add(cw[:], cw[:], cwb[:])
                hw = hbuf.tile([P, ER], F16, tag="hw")
                # hw = (64*h) * 2^-6 * cw -> true h*cw scale; fp16 here so the
                # PE transpose is legal, cast to fp8 on the psum->sbuf copy
                nc.vector.scalar_tensor_tensor(
                    hw.rearrange("p (e r) -> p e r", e=E),
                    ps_h.rearrange("p (e r) -> p e r", e=E),
                    OUT_S,
                    cw[:, :, None].to_broadcast([P, E, R]),
                    op0=Alu.mult, op1=Alu.mult,
                )
                return hw

            def emit_router_pe(t, hw):
                """PE transposes of weighted h + copy back; fills pend[t] hwT."""
                ps_base, ps_h, _ = pend[t]
                ps_t = ps_t_pool.tile([P, ER], F16, tag="t")
                for j in range(4):
                    nc.tensor.transpose(
                        ps_t[:, ts(j, P)], hw[:, ts(j, P)], ident[:]
                    )
                hwT = hbuf.tile([P, 4, P], F8, tag="hwT")
                nc.vector.tensor_copy(hwT.rearrange("p a b -> p (a b)"), ps_t[:])
                pend[t] = (ps_base, ps_h, hwT)

            def emit_router(t):
                emit_router_pe(t, emit_router_dve(t))

            def emit_B_and_out(t):
                """LoRA phase B (fp8 DoubleRow) into base psum, bias, store."""
                ps_base, _, hwT = pend.pop(t)
                for j in range(2):
                    nc.tensor.matmul(
                        ps_base[:, 0:512], hwT[:, ts(j, 2), :],
                        bm_sb[:, ts(j, 2), 0:512],
                        start=False, stop=False,
                        perf_mode=DR, skip_group_check=True,
                    )
                    nc.tensor.matmul(
                        ps_base[:, 512:1024], hwT[:, ts(j, 2), :],
                        bm_sb[:, ts(j, 2), 512:1024],
                        start=False, stop=(j == 1),
                        perf_mode=DR, skip_group_check=True,
                    )
                out_sb = obuf.tile([P, D], F32, tag="out")
                nc.vector.scalar_tensor_tensor(
                    out_sb[:], ps_base[:], OUT_S, b2b_sb[:],
                    op0=Alu.mult, op1=Alu.add,
                )
                nc.scalar.dma_start(y_d[ts(t, P), :], out_sb[:])

            def alloc_psums(t):
                pend[t] = (
                    ps_base_pool.tile([P, D], F32, tag="base", name=f"base{t}"),
                    ps_h_pool.tile([P, ER], F32, tag="h", name=f"h{t}"),
                    None,
                )
                # The shared logits bank must never see start=True (a bank-wide
                # has_written clear would wipe the other tile's half). Instead
                # zero this tile's half; start=False matmuls then accumulate
                # onto 0 (bits set) or overwrite with v (bits clear) — both ok.
                nc.vector.memset(
                    ps_l_shared[:, (t % 2) * 32:(t % 2) * 32 + 32], 0.0
                )
                # ps_h takes only start=False matmuls (DoubleRow), zero it too
                ps_h = pend[t][1]
                nc.vector.memset(ps_h[:], 0.0)
                # on-device x8 cast for this tile (DVE, ~3us, runs in DVE
                # slack well before the tile's first LoRA matmul)
                x16_sb, x8_sb, _ = xts[t]
                nc.vector.tensor_copy(
                    x8_sb.rearrange("p a b -> p (a b)"),
                    x16_sb.rearrange("p a b -> p (a b)"),
                )

            # ---- startup: interleave phase A of tiles 0 and 1 so the PE has
            # two tiles of work while wcat groups stream in ----
            D_OFF = 4
            alloc_psums(0)
            alloc_psums(1)

            for g in range(NPAIR + D_OFF):
                if g < NPAIR:
                    emit_A_pair(0, g, late8=True)
                if g == NPAIR:
                    emit_router(0)
                gg = g - D_OFF
                if 0 <= gg < NPAIR:
                    emit_A_pair(1, gg, late8=True)
                if gg == NPAIR - 4:
                    emit_B_and_out(0)
            hw_pend = {1: emit_router_dve(1)}

            # ---- steady state ----
            for t in range(2, TT - 1):
                if t >= 4:
                    issue_x(t, nc.gpsimd, nc.scalar)
                alloc_psums(t)
                for j in range(NPAIR):
                    emit_A_pair(t, j)
                    if j == 0 and (t - 1) in hw_pend:
                        # previous tile's transposes here: its DVE router
                        # chain is long done, so the PE never stalls on it
                        emit_router_pe(t - 1, hw_pend.pop(t - 1))
                    if j == 8:
                        # previous tile's phase B mid-A so its psum/base slot
                        # frees well before tile t+1 needs it
                        emit_B_and_out(t - 1)
                hw_pend[t] = emit_router_dve(t)

            # ---- last tile: router columns (L, h) stream first so the DVE
            # router chain overlaps the base-column streams; transposes are
            # injected mid-loop -> phase B follows the final matmul directly
            t = TT - 1
            issue_x(t, nc.gpsimd, nc.scalar)
            alloc_psums(t)
            x16_sb, x8_sb, xlo_sb = xts[t]
            ps_base, ps_h, _ = pend[t]
            ps_l = ps_l_shared[:, (t % 2) * 32:(t % 2) * 32 + 32]
            for j in range(NPAIR):
                for k in (2 * j, 2 * j + 1):
                    nc.tensor.matmul(
                        ps_l[:, 0:16], x16_sb[:, k, :], wc(k, D, NW),
                        start=False, stop=False, skip_group_check=True,
                    )
                if j == 2 and (t - 1) in hw_pend:
                    emit_router_pe(t - 1, hw_pend.pop(t - 1))
                if j >= NPAIR // 2:
                    # LoRA bunched late so the on-device x8 cast (DVE) for
                    # this tile completes before its first consumer
                    for jj in (j - NPAIR // 2, j):
                        nc.tensor.matmul(
                            ps_h[:, :], x8_sb[:, ts(jj, 2), :], a8p(jj),
                            start=False, stop=(jj == NPAIR - 1),
                            perf_mode=DR, skip_group_check=True,
                        )
                nc.tensor.matmul(
                    ps_l[:, 16:24], xlo_sb[:, ts(j, 2), :],
                    wg8_sb[:, ts(j, 2), :],
                    start=False, stop=(j == NPAIR - 1),
                    perf_mode=DR, skip_group_check=True,
                )
            hw_last = emit_router_dve(t)
            for k in range(KCH):
                st = k == 0
                nc.tensor.matmul(
                    ps_base[:, 0:512], x16_sb[:, k, :], wc(k, 0, 512),
                    start=st, stop=False, skip_group_check=True,
                )
                nc.tensor.matmul(
                    ps_base[:, 512:1024], x16_sb[:, k, :], wc(k, 512, 1024),
                    start=st, stop=(k == KCH - 1), skip_group_check=True,
                )
                if k == 4:
                    emit_B_and_out(t - 1)
                if k == 10:
                    emit_router_pe(t, hw_last)
            # drain: all 4 phase-B matmuls first (no DVE read interleaved —
            # a mid-stream psum read stalls the remaining matmuls on the
            # tile-granular WAR dependency), then a pipelined 4-way evac so
            # the output DMAs start as early as possible
            ps_base, _, hwT = pend.pop(TT - 1)
            for half in range(2):
                lo, hi = half * 512, half * 512 + 512
                for j in range(2):
                    nc.tensor.matmul(
                        ps_base[:, lo:hi], hwT[:, ts(j, 2), :],
                        bm_sb[:, ts(j, 2), lo:hi],
                        start=False, stop=(j == 1),
                        perf_mode=DR, skip_group_check=True,
                    )
            out_sb = obuf.tile([P, D], F32, tag="out")
            for q in range(4):
                lo, hi = q * 256, q * 256 + 256
                nc.vector.scalar_tensor_tensor(
                    out_sb[:, lo:hi], ps_base[:, lo:hi], OUT_S,
                    b2b_sb[:, lo:hi],
                    op0=Alu.mult, op1=Alu.add,
                )
                nc.scalar.dma_start(
                    y_d[ts(TT - 1, P), lo:hi], out_sb[:, lo:hi]
                )

    nc.finalize()
    return nc


def _prep_shared(Wg, W2, b2, A, Bm, scale):
    """Host-side weight layout prep (replicated across cores)."""
    f16, f32 = np.float16, np.float32
    f8 = ml_dtypes.float8_e4m3

    def chunked(a):
        # [H, N] -> [P, KCH, N]
        return np.ascontiguousarray(
            a.reshape(KCH, P, -1).transpose(1, 0, 2)
        )

    # wcat = [W2*64 | Wg_hi | Wg_lo] fp16
    wg_hi = Wg.astype(f16)
    wg_lo = (Wg.astype(f32) - wg_hi.astype(f32)).astype(f16)
    wcat = np.empty((H, NW), dtype=f16)
    wcat[:, 0:D] = (W2.astype(f32) * W_S).astype(f16)
    wcat[:, D:D + 8] = wg_hi
    wcat[:, D + 8:] = wg_lo
    wcat = chunked(wcat)

    a_flat = np.ascontiguousarray(A.transpose(1, 0, 2)).reshape(H, ER)
    a8 = chunked((a_flat.astype(f32) * W_S).astype(f8))
    wg8 = chunked((Wg.astype(f32) * W_S).astype(f8))

    # Bm with scale and 64x folded, [(e r), d] -> [128, 4, D] fp8
    bms = (Bm.astype(f32) * scale.astype(f32)[:, None, None]).reshape(ER, D)
    bms = np.ascontiguousarray(
        (bms * W_S).reshape(4, P, D).transpose(1, 0, 2)
    ).astype(f8)

    b2b = np.ascontiguousarray(
        np.broadcast_to(b2.astype(f32)[None, :], (P, D))
    )
    return wcat, a8, wg8, bms, b2b


def _prep_x_core(x_c):
    """Per-core x prep: fp16 hi + scaled-fp8 lo; [t, p, k, ti] layout.
    (x8 is derived on-device from x16.)"""
    f32 = np.float32
    f8 = ml_dtypes.float8_e4m3
    x16 = x_c.astype(np.float16)                            # [1024, 4096]
    xlo = ((x_c.astype(f32) - x16.astype(f32)) * XLO_S).astype(f8)

    def lay(a):
        return np.ascontiguousarray(
            a.reshape(TT, P, KCH, P).transpose(0, 3, 2, 1)
        )
    return lay(x16), lay(xlo)


def kernel(x, Wg, W2, b2, A, Bm, scale):
    x = np.asarray(x, dtype=np.float32)
    Wg = np.asarray(Wg, dtype=np.float32)
    W2 = np.asarray(W2, dtype=np.float32)
    b2 = np.asarray(b2, dtype=np.float32)
    A = np.asarray(A, dtype=np.float32)
    Bm = np.asarray(Bm, dtype=np.float32)
    scale = np.asarray(scale, dtype=np.float32)

    if "nc" not in _CACHE:
        _CACHE["nc"] = _build_nc()
    nc = _CACHE["nc"]

    wcat, a8, wg8, bms, b2b = _prep_shared(Wg, W2, b2, A, Bm, scale)
    in_maps = []
    for c in range(NCORES):
        x16, xlo = _prep_x_core(x[c])
        in_maps.append(
            {"x16": x16, "xlo": xlo, "wcat": wcat, "a8": a8,
             "wg8": wg8, "bm": bms, "b2b": b2b}
        )

    res = run_bass_kernel_spmd(nc, in_maps, core_ids=list(range(NCORES)))
    out = np.stack([res.results[c]["y"] for c in range(NCORES)], axis=0)
    return out.astype(np.float32)


# revision 36
# speedup vs baseline: 1.0352x; 1.0120x over previous
"""Trainium2 Bass kernel for DinoVisionTransformer Sparse-MoE FC2 (LoRA experts).

Computation (per token t):
    logits = x @ Wg                      -> top-2 softmax-renormalized weights
    out    = x @ W2 + b2 + sum_e cw[t,e] * scale[e] * (x @ A_e) @ B_e

Sharding: data-parallel over the batch dim (8 batch rows -> 8 NeuronCores,
1024 tokens each). All weights replicated.

Per-core kernel (fp16 base path, fp8e4m3 DoubleRow LoRA path, fp32 PSUM):
  All weight scales are folded by 64 (W2*64 fp16, A*64 fp8, Bm*scale*64 fp8)
  so PSUM accumulates 64*(base + delta); the final DVE pass multiplies by
  2^-6 and adds b2.
  Phase A (per pair of 128-deep k-chunks, contraction over H=4096):
    base: x16 stationary, W2 columns fp16 (2x512 per chunk)
    router hi: x16 @ [Wg_hi | Wg_lo] -> ps_l[0:16] (fp16, fp32 accum)
    LoRA: x8 pair stationary, A8 pair moving, fp8 DoubleRow (2 chunks/instr,
      2 cols/cycle) -> ps_h = 64*h
    router lo: xlo8 (= (x - fp16(x))*4096 in fp8) @ wg8 (=Wg*64 fp8)
      DoubleRow -> ps_l[16:24] = 2^18 * correction
  Router (DVE): logits = reduce(ps_l[0:16]) + 2^-18*ps_l[16:24]; top-2 of 8
    via max8; w1 = sigmoid(l1-l2), w2 = 1-w1; dense cw by equality masks.
  hw8 = fp8(ps_h * cw * 2^-6)  (true h*cw scale), PE-transposed in fp8,
  phase B: 4 fp8 DoubleRow matmuls accumulate 64*delta into ps_base.
  Final: out = ps_base * 2^-6 + b2 (DVE scalar_tensor_tensor), DMA out.
"""

import sys

if "/opt/trn_rl_repo" not in sys.path:
    sys.path.insert(0, "/opt/trn_rl_repo")

import numpy as np
import ml_dtypes

import concourse.bass as bass  # noqa: F401  (registers types)
import concourse.mybir as mybir
import concourse.tile as tile
from concourse import bacc
from concourse.bass import ts
from concourse.bass_utils import run_bass_kernel_spmd
from concourse.masks import make_identity

P = 128
KCH = 32          # H / 128 contraction chunks
NPAIR = 16        # DoubleRow k-chunk pairs per tile
TT = 8            # 128-token tiles per core
H = 4096
D = 1024
E = 8
R = 64
ER = E * R        # 512
NW = D + 8 + 8    # 1040 fp16 wcat columns: [W2*64 | Wg_hi | Wg_lo]
NCORES = 8
XLO_S = 4096.0    # host scale on xlo before fp8 quantization
W_S = 64.0        # host scale on W2/A/Bm before quantization
CORR_S = 1.0 / (XLO_S * W_S)   # ps_l[16:24] -> logit units
OUT_S = 1.0 / W_S              # ps_base -> output units

F8 = mybir.dt.float8e4
F16 = mybir.dt.float16
F32 = mybir.dt.float32
DR = mybir.MatmulPerfMode.DoubleRow

_CACHE = {}


def _build_nc():
    nc = bacc.Bacc("TRN2")

    x16_d = nc.dram_tensor("x16", [TT, P, KCH, P], F16, kind="ExternalInput")
    xlo_d = nc.dram_tensor("xlo", [TT, P, KCH, P], F8, kind="ExternalInput")
    wcat_d = nc.dram_tensor("wcat", [P, KCH, NW], F16, kind="ExternalInput")
    a8_d = nc.dram_tensor("a8", [P, KCH, ER], F8, kind="ExternalInput")
    wg8_d = nc.dram_tensor("wg8", [P, KCH, 8], F8, kind="ExternalInput")
    bm_d = nc.dram_tensor("bm", [P, 4, D], F8, kind="ExternalInput")
    b2b_d = nc.dram_tensor("b2b", [P, D], F32, kind="ExternalInput")
    y_d = nc.dram_tensor("y", [TT * P, D], F32, kind="ExternalOutput")

    Sig = mybir.ActivationFunctionType.Sigmoid
    Alu = mybir.AluOpType

    with tile.TileContext(nc) as tc:
        with (
            tc.tile_pool(name="wres", bufs=1) as wres,
            tc.tile_pool(name="xin", bufs=3) as xin,
            tc.tile_pool(name="small", bufs=2) as small,
            tc.tile_pool(name="hbuf", bufs=2) as hbuf,
            tc.tile_pool(name="obuf", bufs=2) as obuf,
            tc.tile_pool(name="ps_base", bufs=2, space="PSUM") as ps_base_pool,
            tc.tile_pool(name="ps_h", bufs=2, space="PSUM") as ps_h_pool,
            tc.tile_pool(name="ps_l", bufs=1, space="PSUM") as ps_l_pool,
            tc.tile_pool(name="ps_t", bufs=1, space="PSUM") as ps_t_pool,
        ):
            # ---- startup DMA. The DMA engines share bandwidth round-robin
            # across outstanding transfers, so the critical-path streams
            # (x16 of tile 0, wcat groups) are issued FIRST from sync (the
            # empirically fast path), while everything needed only from
            # mid-tile-0 onward (xlo, a8 m1+, wg8, bm, b2b) is issued from
            # the vector engine AFTER the x8 cast of tile 0 — a real
            # dependency that keeps those transfers out of the early
            # bandwidth fight. ----
            xts = {}
            late_batches = {0: [], 1: [], 2: []}  # drained after cast(t)

            def issue_x(t0, x16_eng, xlo_eng):
                # x8 is not streamed: it is cast on-device from x16 (saves
                # 4.2 MB of HBM traffic). Cast is emitted in alloc_psums.
                x16_ = xin.tile([P, KCH, P], F16, tag="x16")
                x8_ = xin.tile([P, KCH, P], F8, tag="x8")
                xlo_ = xin.tile([P, KCH, P], F8, tag="xlo")
                if x16_eng is not None:
                    x16_eng.dma_start(x16_[:], x16_d[t0])
                if xlo_eng is not None:
                    xlo_eng.dma_start(xlo_[:], xlo_d[t0])
                xts[t0] = (x16_, x8_, xlo_)
                return x16_, xlo_

            ident = wres.tile([P, P], F16, tag="ident")
            make_identity(nc, ident[:])
            _, xlo0 = issue_x(0, nc.sync, None)
            wcat_sb = []
            a8_sb = []
            wcat0_ = wres.tile([P, 4, NW], F16, tag="wcat0", name="wcat0")
            wcat_sb.append(wcat0_)
            nc.sync.dma_start(wcat0_[:], wcat_d[:, ts(0, 4), :])
            _, xlo1 = issue_x(1, nc.sync, None)
            for g in range(1, 8):
                t_ = wres.tile([P, 4, NW], F16, tag=f"wcat{g}")
                nc.sync.dma_start(t_[:], wcat_d[:, ts(g, 4), :])
                wcat_sb.append(t_)
                if g == 3:
                    a_ = wres.tile([P, 8, ER], F8, tag="a80")
                    nc.sync.dma_start(a_[:], a8_d[:, ts(0, 8), :])
                    a8_sb.append(a_)
            wg8_sb = wres.tile([P, KCH, 8], F8, tag="wg8")
            bm_sb = wres.tile([P, 4, D], F8, tag="bm")
            b2b_sb = wres.tile([P, D], F32, tag="b2b")
            for m in range(1, 4):
                a_ = wres.tile([P, 8, ER], F8, tag=f"a8{m}", name=f"a8{m}")
                a8_sb.append(a_)
            x16_2, xlo_2 = issue_x(2, None, None)
            x16_3, xlo_3 = issue_x(3, None, None)
            # deferred issues, in consumption order, drained on gpsimd after
            # each early cast — so their transfers start only once the
            # critical tile-0/1 x16 + wcat stream has landed
            late_batches[0] = [
                (xlo0[:], xlo_d[0]),
                (wg8_sb[:], wg8_d[:]),
                (a8_sb[1][:], a8_d[:, ts(1, 8), :]),
                (a8_sb[2][:], a8_d[:, ts(2, 8), :]),
                (a8_sb[3][:], a8_d[:, ts(3, 8), :]),
            ]
            late_batches[1] = [
                (xlo1[:], xlo_d[1]),
                (bm_sb[:], bm_d[:]),
                (b2b_sb[:], b2b_d[:]),
                (x16_2[:], x16_d[2]),
                (xlo_2[:], xlo_d[2]),
            ]
            late_batches[2] = [
                (x16_3[:], x16_d[3]),
                (xlo_3[:], xlo_d[3]),
            ]
            # drain the deferred batches on gpsimd, each gated behind a tiny
            # copy that reads the corresponding x16 tile — so these
            # transfers start only after the critical early streams landed.
            # Batch 2 needs no gate: its destinations reuse tile-0 xin
            # buffers, so the WAR dependency throttles them naturally.
            for gi in (0, 1):
                gate_ = small.tile([P, 8], F16, tag="gate", name=f"gate{gi}")
                nc.gpsimd.tensor_copy(gate_[:], xts[gi][0][:, 0, 0:8])
                for dst, src in late_batches.pop(gi):
                    nc.gpsimd.dma_start(dst, src)
            for dst, src in late_batches.pop(2):
                nc.gpsimd.dma_start(dst, src)

            def wc(k, lo, hi):
                return wcat_sb[k // 4][:, k % 4, lo:hi]

            def a8p(j):
                return a8_sb[j // 4][:, (j % 4) * 2:(j % 4) * 2 + 2, :]

            # shared logits psum bank: tile t uses half (t % 2).
            # cols [0:16] = x16 @ [Wg_hi | Wg_lo]; cols [16:24] = 2^18 x the
            # xlo correction (fp8 DoubleRow; rescaled on the DVE afterwards)
            ps_l_shared = ps_l_pool.tile([P, 64], F32, tag="l")

            pend = {}   # t -> (ps_base, ps_h, hwT or None)

            def emit_A_pair(t, j, late8=False, warm_only=False):
                """Phase-A matmuls for k-chunk pair j (chunks 2j, 2j+1).

                late8: bunch the fp8 LoRA + xlo-correction DoubleRow matmuls
                into the second half of the pair loop (two per slot) so the
                fp8 x streams can be issued after the first wcat groups."""
                x16_sb, x8_sb, xlo_sb = xts[t]
                ps_base, ps_h, _ = pend[t]
                ps_l = ps_l_shared[:, (t % 2) * 32:(t % 2) * 32 + 32]

                def lora(jj):
                    nc.tensor.matmul(
                        ps_h[:, :], x8_sb[:, ts(jj, 2), :], a8p(jj),
                        start=False, stop=(jj == NPAIR - 1),
                        perf_mode=DR, skip_group_check=True,
                    )

                def xcorr(jj):
                    nc.tensor.matmul(
                        ps_l[:, 16:24], xlo_sb[:, ts(jj, 2), :],
                        wg8_sb[:, ts(jj, 2), :],
                        start=False, stop=(jj == NPAIR - 1),
                        perf_mode=DR, skip_group_check=True,
                    )

                for k in (2 * j, 2 * j + 1):
                    st = k == 0
                    # order: tiny-N matmuls sit between 512-col streams so
                    # their self-loading weight fetches hide under the streams
                    nc.tensor.matmul(
                        ps_base[:, 0:512], x16_sb[:, k, :], wc(k, 0, 512),
                        start=st, stop=False, skip_group_check=True,
                    )
                    if not warm_only:
                        nc.tensor.matmul(
                            ps_l[:, 0:16], x16_sb[:, k, :], wc(k, D, NW),
                            start=False, stop=False, skip_group_check=True,
                        )
                    nc.tensor.matmul(
                        ps_base[:, 512:1024], x16_sb[:, k, :], wc(k, 512, 1024),
                        start=st, stop=False, skip_group_check=True,
                    )
                    if warm_only:
                        continue
                    if k % 2 == 1:
                        if late8:
                            if j >= NPAIR // 2:
                                for jj in (j - NPAIR // 2, j):
                                    lora(jj)
                                    xcorr(jj)
                        else:
                            lora(j)
                            xcorr(j)

            def emit_router_dve(t):
                """Router math + h-weighting (DVE/ACT only); returns hw8."""
                ps_base, ps_h, _ = pend[t]
                ps_l = ps_l_shared[:, (t % 2) * 32:(t % 2) * 32 + 32]
                logits = small.tile([P, 8], F32, tag="logits")
                nc.vector.tensor_reduce(
                    logits[:],
                    ps_l[:, 0:16].rearrange("p (s j) -> p j s", s=2),
                    axis=mybir.AxisListType.X,
                    op=Alu.add,
                )
                nc.vector.scalar_tensor_tensor(
                    logits[:], ps_l[:, 16:24], CORR_S, logits[:],
                    op0=Alu.mult, op1=Alu.add,
                )
                m8 = small.tile([P, 8], F32, tag="m8")
                nc.vector.max(m8[:], logits[:])
                g_ = small.tile([P, 1], F32, tag="gap")
                nc.vector.tensor_sub(g_[:], m8[:, 0:1], m8[:, 1:2])
                w1 = small.tile([P, 1], F32, tag="w1")
                nc.scalar.activation(w1[:], g_[:], Sig)
                w2 = small.tile([P, 1], F32, tag="w2")
                nc.scalar.activation(w2[:], g_[:], Sig, scale=-1.0)
                cw = small.tile([P, 8], F32, tag="cw")
                cwb = small.tile([P, 8], F32, tag="cwb")
                nc.vector.scalar_tensor_tensor(
                    cw[:], logits[:], m8[:, 0:1], w1[:, 0:1].to_broadcast([P, 8]),
                    op0=Alu.is_equal, op1=Alu.mult,
                )
                nc.vector.scalar_tensor_tensor(
                    cwb[:], logits[:], m8[:, 1:2], w2[:, 0:1].to_broadcast([P, 8]),
                    op0=Alu.is_equal, op1=Alu.mult,
                )
                nc.vector.tensor_add(cw[:], cw[:], cwb[:])
                hw = hbuf.tile([P, ER], F16, tag="hw")
                # hw = (64*h) * 2^-6 * cw -> true h*cw scale; fp16 here so the
                # PE transpose is legal, cast to fp8 on the psum->sbuf copy
                nc.vector.scalar_tensor_tensor(
                    hw.rearrange("p (e r) -> p e r", e=E),
                    ps_h.rearrange("p (e r) -> p e r", e=E),
                    OUT_S,
                    cw[:, :, None].to_broadcast([P, E, R]),
                    op0=Alu.mult, op1=Alu.mult,
                )
                return hw

            def emit_router_pe(t, hw):
                """PE transposes of weighted h + copy back; fills pend[t] hwT."""
                ps_base, ps_h, _ = pend[t]
                ps_t = ps_t_pool.tile([P, ER], F16, tag="t")
                for j in range(4):
                    nc.tensor.transpose(
                        ps_t[:, ts(j, P)], hw[:, ts(j, P)], ident[:]
                    )
                hwT = hbuf.tile([P, 4, P], F8, tag="hwT")
                nc.vector.tensor_copy(hwT.rearrange("p a b -> p (a b)"), ps_t[:])
                pend[t] = (ps_base, ps_h, hwT)

            def emit_router(t):
                emit_router_pe(t, emit_router_dve(t))

            def emit_B_and_out(t):
                """LoRA phase B (fp8 DoubleRow) into base psum, bias, store."""
                ps_base, _, hwT = pend.pop(t)
                for j in range(2):
                    nc.tensor.matmul(
                        ps_base[:, 0:512], hwT[:, ts(j, 2), :],
                        bm_sb[:, ts(j, 2), 0:512],
                        start=False, stop=False,
                        perf_mode=DR, skip_group_check=True,
                    )
                    nc.tensor.matmul(
                        ps_base[:, 512:1024], hwT[:, ts(j, 2), :],
                        bm_sb[:, ts(j, 2), 512:1024],
                        start=False, stop=(j == 1),
                        perf_mode=DR, skip_group_check=True,
                    )
                out_sb = obuf.tile([P, D], F32, tag="out")
                nc.vector.scalar_tensor_tensor(
                    out_sb[:], ps_base[:], OUT_S, b2b_sb[:],
                    op0=Alu.mult, op1=Alu.add,
                )
                nc.scalar.dma_start(y_d[ts(t, P), :], out_sb[:])

            def alloc_psums(t):
                pend[t] = (
                    ps_base_pool.tile([P, D], F32, tag="base", name=f"base{t}"),
                    ps_h_pool.tile([P, ER], F32, tag="h", name=f"h{t}"),
                    None,
                )
                # The shared logits bank must never see start=True (a bank-wide
                # has_written clear would wipe the other tile's half). Instead
                # zero this tile's half; start=False matmuls then accumulate
                # onto 0 (bits set) or overwrite with v (bits clear) — both ok.
                nc.vector.memset(
                    ps_l_shared[:, (t % 2) * 32:(t % 2) * 32 + 32], 0.0
                )
                # ps_h takes only start=False matmuls (DoubleRow), zero it too
                ps_h = pend[t][1]
                nc.vector.memset(ps_h[:], 0.0)
                # on-device x8 cast for this tile (DVE, ~3us, runs in DVE
                # slack well before the tile's first LoRA matmul)
                x16_sb, x8_sb, _ = xts[t]
                nc.vector.tensor_copy(
                    x8_sb.rearrange("p a b -> p (a b)"),
                    x16_sb.rearrange("p a b -> p (a b)"),
                )

            # ---- startup: interleave phase A of tiles 0 and 1 so the PE has
            # two tiles of work while wcat groups stream in ----
            D_OFF = 4
            alloc_psums(0)
            alloc_psums(1)

            for g in range(NPAIR + D_OFF):
                if g < NPAIR:
                    emit_A_pair(0, g, late8=True)
                if g == NPAIR:
                    emit_router(0)
                gg = g - D_OFF
                if 0 <= gg < NPAIR:
                    emit_A_pair(1, gg, late8=True)
                if gg == NPAIR - 4:
                    emit_B_and_out(0)
            hw_pend = {1: emit_router_dve(1)}

            # ---- steady state ----
            for t in range(2, TT - 1):
                if t >= 4:
                    issue_x(t, nc.gpsimd, nc.scalar)
                alloc_psums(t)
                for j in range(NPAIR):
                    emit_A_pair(t, j)
                    if j == 0 and (t - 1) in hw_pend:
                        # previous tile's transposes here: its DVE router
                        # chain is long done, so the PE never stalls on it
                        emit_router_pe(t - 1, hw_pend.pop(t - 1))
                    if j == 8:
                        # previous tile's phase B mid-A so its psum/base slot
                        # frees well before tile t+1 needs it
                        emit_B_and_out(t - 1)
                hw_pend[t] = emit_router_dve(t)

            # ---- last tile: router columns (L, h) stream first so the DVE
            # router chain overlaps the base-column streams; transposes are
            # injected mid-loop -> phase B follows the final matmul directly
            t = TT - 1
            issue_x(t, nc.gpsimd, nc.scalar)
            alloc_psums(t)
            x16_sb, x8_sb, xlo_sb = xts[t]
            ps_base, ps_h, _ = pend[t]
            ps_l = ps_l_shared[:, (t % 2) * 32:(t % 2) * 32 + 32]
            for j in range(NPAIR):
                for k in (2 * j, 2 * j + 1):
                    nc.tensor.matmul(
                        ps_l[:, 0:16], x16_sb[:, k, :], wc(k, D, NW),
                        start=False, stop=False, skip_group_check=True,
                    )
                if j == 2 and (t - 1) in hw_pend:
                    emit_router_pe(t - 1, hw_pend.pop(t - 1))
                if j >= NPAIR // 2:
                    # LoRA bunched late so the on-device x8 cast (DVE) for
                    # this tile completes before its first consumer
                    for jj in (j - NPAIR // 2, j):
                        nc.tensor.matmul(
                            ps_h[:, :], x8_sb[:, ts(jj, 2), :], a8p(jj),
                            start=False, stop=(jj == NPAIR - 1),
                            perf_mode=DR, skip_group_check=True,
                        )
                nc.tensor.matmul(
                    ps_l[:, 16:24], xlo_sb[:, ts(j, 2), :],
                    wg8_sb[:, ts(j, 2), :],
                    start=False, stop=(j == NPAIR - 1),
                    perf_mode=DR, skip_group_check=True,
                )
            hw_last = emit_router_dve(t)
            for k in range(KCH):
                st = k == 0
                nc.tensor.matmul(
                    ps_base[:, 0:512], x16_sb[:, k, :], wc(k, 0, 512),
                    start=st, stop=False, skip_group_check=True,
                )
                nc.tensor.matmul(
                    ps_base[:, 512:1024], x16_sb[:, k, :], wc(k, 512, 1024),
                    start=st, stop=(k == KCH - 1), skip_group_check=True,
                )
                if k == 4:
                    emit_B_and_out(t - 1)
                if k == 10:
                    emit_router_pe(t, hw_last)
            # drain: all 4 phase-B matmuls first (no DVE read interleaved —
            # a mid-stream psum read stalls the remaining matmuls on the
            # tile-granular WAR dependency), then a pipelined 4-way evac so
            # the output DMAs start as early as possible
            ps_base, _, hwT = pend.pop(TT - 1)
            for half in range(2):
                lo, hi = half * 512, half * 512 + 512
                for j in range(2):
                    nc.tensor.matmul(
                        ps_base[:, lo:hi], hwT[:, ts(j, 2), :],
                        bm_sb[:, ts(j, 2), lo:hi],
                        start=False, stop=(j == 1),
                        perf_mode=DR, skip_group_check=True,
                    )
            out_sb = obuf.tile([P, D], F32, tag="out")
            for q in range(4):
                lo, hi = q * 256, q * 256 + 256
                nc.vector.scalar_tensor_tensor(
                    out_sb[:, lo:hi], ps_base[:, lo:hi], OUT_S,
                    b2b_sb[:, lo:hi],
                    op0=Alu.mult, op1=Alu.add,
                )
                nc.scalar.dma_start(
                    y_d[ts(TT - 1, P), lo:hi], out_sb[:, lo:hi]
                )

    nc.finalize()
    return nc


def _prep_shared(Wg, W2, b2, A, Bm, scale):
    """Host-side weight layout prep (replicated across cores)."""
    f16, f32 = np.float16, np.float32
    f8 = ml_dtypes.float8_e4m3

    def chunked(a):
        # [H, N] -> [P, KCH, N]
        return np.ascontiguousarray(
            a.reshape(KCH, P, -1).transpose(1, 0, 2)
        )

    # wcat = [W2*64 | Wg_hi | Wg_lo] fp16
    wg_hi = Wg.astype(f16)
    wg_lo = (Wg.astype(f32) - wg_hi.astype(f32)).astype(f16)
    wcat = np.empty((H, NW), dtype=f16)
    wcat[:, 0:D] = (W2.astype(f32) * W_S).astype(f16)
    wcat[:, D:D + 8] = wg_hi
    wcat[:, D + 8:] = wg_lo
    wcat = chunked(wcat)

    a_flat = np.ascontiguousarray(A.transpose(1, 0, 2)).reshape(H, ER)
    a8 = chunked((a_flat.astype(f32) * W_S).astype(f8))
    wg8 = chunked((Wg.astype(f32) * W_S).astype(f8))

    # Bm with scale and 64x folded, [(e r), d] -> [128, 4, D] fp8
    bms = (Bm.astype(f32) * scale.astype(f32)[:, None, None]).reshape(ER, D)
    bms = np.ascontiguousarray(
        (bms * W_S).reshape(4, P, D).transpose(1, 0, 2)
    ).astype(f8)

    b2b = np.ascontiguousarray(
        np.broadcast_to(b2.astype(f32)[None, :], (P, D))
    )
    return wcat, a8, wg8, bms, b2b


def _prep_x_core(x_c):
    """Per-core x prep: fp16 hi + scaled-fp8 lo; [t, p, k, ti] layout.
    (x8 is derived on-device from x16.)"""
    f32 = np.float32
    f8 = ml_dtypes.float8_e4m3
    x16 = x_c.astype(np.float16)                            # [1024, 4096]
    xlo = ((x_c.astype(f32) - x16.astype(f32)) * XLO_S).astype(f8)

    def lay(a):
        return np.ascontiguousarray(
            a.reshape(TT, P, KCH, P).transpose(0, 3, 2, 1)
        )
    return lay(x16), lay(xlo)


def kernel(x, Wg, W2, b2, A, Bm, scale):
    x = np.asarray(x, dtype=np.float32)
    Wg = np.asarray(Wg, dtype=np.float32)
    W2 = np.asarray(W2, dtype=np.float32)
    b2 = np.asarray(b2, dtype=np.float32)
    A = np.asarray(A, dtype=np.float32)
    Bm = np.asarray(Bm, dtype=np.float32)
    scale = np.asarray(scale, dtype=np.float32)

    if "nc" not in _CACHE:
        _CACHE["nc"] = _build_nc()
    nc = _CACHE["nc"]

    wcat, a8, wg8, bms, b2b = _prep_shared(Wg, W2, b2, A, Bm, scale)
    in_maps = []
    for c in range(NCORES):
        x16, xlo = _prep_x_core(x[c])
        in_maps.append(
            {"x16": x16, "xlo": xlo, "wcat": wcat, "a8": a8,
             "wg8": wg8, "bm": bms, "b2b": b2b}
        )

    res = run_bass_kernel_spmd(nc, in_maps, core_ids=list(range(NCORES)))
    out = np.stack([res.results[c]["y"] for c in range(NCORES)], axis=0)
    return out.astype(np.float32)


# revision 40
# speedup vs baseline: 1.0411x; 1.0057x over previous
"""Trainium2 Bass kernel for DinoVisionTransformer Sparse-MoE FC2 (LoRA experts).

Computation (per token t):
    logits = x @ Wg                      -> top-2 softmax-renormalized weights
    out    = x @ W2 + b2 + sum_e cw[t,e] * scale[e] * (x @ A_e) @ B_e

Sharding: data-parallel over the batch dim (8 batch rows -> 8 NeuronCores,
1024 tokens each). All weights replicated.

Per-core kernel (fp16 base path, fp8e4m3 DoubleRow LoRA path, fp32 PSUM):
  All weight scales are folded by 64 (W2*64 fp16, A*64 fp8, Bm*scale*64 fp8)
  so PSUM accumulates 64*(base + delta); the final DVE pass multiplies by
  2^-6 and adds b2.
  Phase A (per pair of 128-deep k-chunks, contraction over H=4096):
    base: x16 stationary, W2 columns fp16 (2x512 per chunk)
    router hi: x16 @ [Wg_hi | Wg_lo] -> ps_l[0:16] (fp16, fp32 accum)
    LoRA: x8 pair stationary, A8 pair moving, fp8 DoubleRow (2 chunks/instr,
      2 cols/cycle) -> ps_h = 64*h
    router lo: xlo8 (= (x - fp16(x))*4096 in fp8) @ wg8 (=Wg*64 fp8)
      DoubleRow -> ps_l[16:24] = 2^18 * correction
  Router (DVE): logits = reduce(ps_l[0:16]) + 2^-18*ps_l[16:24]; top-2 of 8
    via max8; w1 = sigmoid(l1-l2), w2 = 1-w1; dense cw by equality masks.
  hw8 = fp8(ps_h * cw * 2^-6)  (true h*cw scale), PE-transposed in fp8,
  phase B: 4 fp8 DoubleRow matmuls accumulate 64*delta into ps_base.
  Final: out = ps_base * 2^-6 + b2 (DVE scalar_tensor_tensor), DMA out.
"""

import sys

if "/opt/trn_rl_repo" not in sys.path:
    sys.path.insert(0, "/opt/trn_rl_repo")

import numpy as np
import ml_dtypes

import concourse.bass as bass  # noqa: F401  (registers types)
import concourse.mybir as mybir
import concourse.tile as tile
from concourse import bacc
from concourse.bass import ts
from concourse.bass_utils import run_bass_kernel_spmd
from concourse.masks import make_identity

P = 128
KCH = 32          # H / 128 contraction chunks
NPAIR = 16        # DoubleRow k-chunk pairs per tile
TT = 8            # 128-token tiles per core
H = 4096
D = 1024
E = 8
R = 64
ER = E * R        # 512
NW = D + 8 + 8    # 1040 fp16 wcat columns: [W2*64 | Wg_hi | Wg_lo]
NCORES = 8
XLO_S = 4096.0    # host scale on xlo before fp8 quantization
W_S = 64.0        # host scale on W2/A/Bm before quantization
CORR_S = 1.0 / (XLO_S * W_S)   # ps_l[16:24] -> logit units
OUT_S = 1.0 / W_S              # ps_base -> output units

F8 = mybir.dt.float8e4
F16 = mybir.dt.float16
F32 = mybir.dt.float32
DR = mybir.MatmulPerfMode.DoubleRow

_CACHE = {}


def _build_nc():
    nc = bacc.Bacc("TRN2")

    x16_d = nc.dram_tensor("x16", [TT, P, KCH, P], F16, kind="ExternalInput")
    xlo_d = nc.dram_tensor("xlo", [TT, P, KCH, P], F8, kind="ExternalInput")
    wcat_d = nc.dram_tensor("wcat", [P, KCH, NW], F16, kind="ExternalInput")
    a8_d = nc.dram_tensor("a8", [P, KCH, ER], F8, kind="ExternalInput")
    wg8_d = nc.dram_tensor("wg8", [P, KCH, 8], F8, kind="ExternalInput")
    bm_d = nc.dram_tensor("bm", [P, 4, D], F8, kind="ExternalInput")
    b2b_d = nc.dram_tensor("b2b", [P, D], F32, kind="ExternalInput")
    y_d = nc.dram_tensor("y", [TT * P, D], F32, kind="ExternalOutput")

    Sig = mybir.ActivationFunctionType.Sigmoid
    Alu = mybir.AluOpType

    with tile.TileContext(nc) as tc:
        with (
            tc.tile_pool(name="wres", bufs=1) as wres,
            tc.tile_pool(name="xin", bufs=4) as xin,
            tc.tile_pool(name="small", bufs=2) as small,
            tc.tile_pool(name="hbuf", bufs=2) as hbuf,
            tc.tile_pool(name="obuf", bufs=2) as obuf,
            tc.tile_pool(name="ps_base", bufs=2, space="PSUM") as ps_base_pool,
            tc.tile_pool(name="ps_h", bufs=2, space="PSUM") as ps_h_pool,
            tc.tile_pool(name="ps_l", bufs=1, space="PSUM") as ps_l_pool,
            tc.tile_pool(name="ps_t", bufs=1, space="PSUM") as ps_t_pool,
        ):
            # ---- startup DMA. The DMA engines share bandwidth round-robin
            # across outstanding transfers, so the critical-path streams
            # (x16 of tile 0, wcat groups) are issued FIRST from sync (the
            # empirically fast path), while everything needed only from
            # mid-tile-0 onward (xlo, a8 m1+, wg8, bm, b2b) is issued from
            # the vector engine AFTER the x8 cast of tile 0 — a real
            # dependency that keeps those transfers out of the early
            # bandwidth fight. ----
            xts = {}
            late_batches = {0: [], 1: [], 2: []}  # drained after cast(t)

            def issue_x(t0, x16_eng, xlo_eng):
                # x8 is not streamed: it is cast on-device from x16 (saves
                # 4.2 MB of HBM traffic). Cast is emitted in alloc_psums.
                x16_ = xin.tile([P, KCH, P], F16, tag="x16")
                x8_ = xin.tile([P, KCH, P], F8, tag="x8")
                xlo_ = xin.tile([P, KCH, P], F8, tag="xlo")
                if x16_eng is not None:
                    x16_eng.dma_start(x16_[:], x16_d[t0])
                if xlo_eng is not None:
                    xlo_eng.dma_start(xlo_[:], xlo_d[t0])
                xts[t0] = (x16_, x8_, xlo_)
                return x16_, xlo_

            ident = wres.tile([P, P], F16, tag="ident")
            make_identity(nc, ident[:])
            _, xlo0 = issue_x(0, nc.sync, None)
            wcat_sb = []
            a8_sb = []
            wcat0_ = wres.tile([P, 4, NW], F16, tag="wcat0", name="wcat0")
            wcat_sb.append(wcat0_)
            nc.sync.dma_start(wcat0_[:], wcat_d[:, ts(0, 4), :])
            _, xlo1 = issue_x(1, nc.sync, None)
            for g in range(1, 8):
                t_ = wres.tile([P, 4, NW], F16, tag=f"wcat{g}")
                nc.sync.dma_start(t_[:], wcat_d[:, ts(g, 4), :])
                wcat_sb.append(t_)
                if g == 3:
                    a_ = wres.tile([P, 8, ER], F8, tag="a80")
                    nc.sync.dma_start(a_[:], a8_d[:, ts(0, 8), :])
                    a8_sb.append(a_)
            wg8_sb = wres.tile([P, KCH, 8], F8, tag="wg8")
            bm_sb = wres.tile([P, 4, D], F8, tag="bm")
            b2b_sb = wres.tile([P, D], F32, tag="b2b")
            for m in range(1, 4):
                a_ = wres.tile([P, 8, ER], F8, tag=f"a8{m}", name=f"a8{m}")
                a8_sb.append(a_)
            x16_2, xlo_2 = issue_x(2, None, None)
            x16_3, xlo_3 = issue_x(3, None, None)
            # deferred issues, in consumption order, drained on gpsimd after
            # each early cast — so their transfers start only once the
            # critical tile-0/1 x16 + wcat stream has landed
            late_batches[0] = [
                (xlo0[:], xlo_d[0]),
                (wg8_sb[:], wg8_d[:]),
                (a8_sb[1][:], a8_d[:, ts(1, 8), :]),
                (a8_sb[2][:], a8_d[:, ts(2, 8), :]),
                (a8_sb[3][:], a8_d[:, ts(3, 8), :]),
            ]
            late_batches[1] = [
                (xlo1[:], xlo_d[1]),
                (bm_sb[:], bm_d[:]),
                (b2b_sb[:], b2b_d[:]),
                (x16_2[:], x16_d[2]),
                (xlo_2[:], xlo_d[2]),
            ]
            late_batches[2] = [
                (x16_3[:], x16_d[3]),
                (xlo_3[:], xlo_d[3]),
            ]
            # drain the deferred batches on gpsimd, each gated behind a tiny
            # copy that reads the corresponding x16 tile — so these
            # transfers start only after the critical early streams landed.
            # Batch 2 needs no gate: its destinations reuse tile-0 xin
            # buffers, so the WAR dependency throttles them naturally.
            for gi in (0, 1):
                gate_ = small.tile([P, 8], F16, tag="gate", name=f"gate{gi}")
                nc.gpsimd.tensor_copy(gate_[:], xts[gi][0][:, 0, 0:8])
                for dst, src in late_batches.pop(gi):
                    nc.gpsimd.dma_start(dst, src)
            for dst, src in late_batches.pop(2):
                nc.gpsimd.dma_start(dst, src)

            def wc(k, lo, hi):
                return wcat_sb[k // 4][:, k % 4, lo:hi]

            def a8p(j):
                return a8_sb[j // 4][:, (j % 4) * 2:(j % 4) * 2 + 2, :]

            # shared logits psum bank: tile t uses half (t % 2).
            # cols [0:16] = x16 @ [Wg_hi | Wg_lo]; cols [16:24] = 2^18 x the
            # xlo correction (fp8 DoubleRow; rescaled on the DVE afterwards)
            ps_l_shared = ps_l_pool.tile([P, 64], F32, tag="l")

            pend = {}   # t -> (ps_base, ps_h, hwT or None)

            def emit_A_pair(t, j, late8=False, warm_only=False):
                """Phase-A matmuls for k-chunk pair j (chunks 2j, 2j+1).

                late8: bunch the fp8 LoRA + xlo-correction DoubleRow matmuls
                into the second half of the pair loop (two per slot) so the
                fp8 x streams can be issued after the first wcat groups."""
                x16_sb, x8_sb, xlo_sb = xts[t]
                ps_base, ps_h, _ = pend[t]
                ps_l = ps_l_shared[:, (t % 2) * 32:(t % 2) * 32 + 32]

                def lora(jj):
                    nc.tensor.matmul(
                        ps_h[:, :], x8_sb[:, ts(jj, 2), :], a8p(jj),
                        start=False, stop=(jj == NPAIR - 1),
                        perf_mode=DR, skip_group_check=True,
                    )

                def xcorr(jj):
                    nc.tensor.matmul(
                        ps_l[:, 16:24], xlo_sb[:, ts(jj, 2), :],
                        wg8_sb[:, ts(jj, 2), :],
                        start=False, stop=(jj == NPAIR - 1),
                        perf_mode=DR, skip_group_check=True,
                    )

                for k in (2 * j, 2 * j + 1):
                    st = k == 0
                    # order: tiny-N matmuls sit between 512-col streams so
                    # their self-loading weight fetches hide under the streams
                    nc.tensor.matmul(
                        ps_base[:, 0:512], x16_sb[:, k, :], wc(k, 0, 512),
                        start=st, stop=False, skip_group_check=True,
                    )
                    if not warm_only:
                        nc.tensor.matmul(
                            ps_l[:, 0:16], x16_sb[:, k, :], wc(k, D, NW),
                            start=False, stop=False, skip_group_check=True,
                        )
                    nc.tensor.matmul(
                        ps_base[:, 512:1024], x16_sb[:, k, :], wc(k, 512, 1024),
                        start=st, stop=False, skip_group_check=True,
                    )
                    if warm_only:
                        continue
                    if k % 2 == 1:
                        if late8:
                            if j >= NPAIR // 2:
                                for jj in (j - NPAIR // 2, j):
                                    lora(jj)
                                    xcorr(jj)
                        else:
                            lora(j)
                            xcorr(j)

            def emit_router_dve(t):
                """Router math + h-weighting (DVE/ACT only); returns hw8."""
                ps_base, ps_h, _ = pend[t]
                ps_l = ps_l_shared[:, (t % 2) * 32:(t % 2) * 32 + 32]
                logits = small.tile([P, 8], F32, tag="logits")
                nc.vector.tensor_reduce(
                    logits[:],
                    ps_l[:, 0:16].rearrange("p (s j) -> p j s", s=2),
                    axis=mybir.AxisListType.X,
                    op=Alu.add,
                )
                nc.vector.scalar_tensor_tensor(
                    logits[:], ps_l[:, 16:24], CORR_S, logits[:],
                    op0=Alu.mult, op1=Alu.add,
                )
                m8 = small.tile([P, 8], F32, tag="m8")
                nc.vector.max(m8[:], logits[:])
                g_ = small.tile([P, 1], F32, tag="gap")
                nc.vector.tensor_sub(g_[:], m8[:, 0:1], m8[:, 1:2])
                w1 = small.tile([P, 1], F32, tag="w1")
                nc.scalar.activation(w1[:], g_[:], Sig)
                w2 = small.tile([P, 1], F32, tag="w2")
                nc.scalar.activation(w2[:], g_[:], Sig, scale=-1.0)
                cw = small.tile([P, 8], F32, tag="cw")
                cwb = small.tile([P, 8], F32, tag="cwb")
                nc.vector.scalar_tensor_tensor(
                    cw[:], logits[:], m8[:, 0:1], w1[:, 0:1].to_broadcast([P, 8]),
                    op0=Alu.is_equal, op1=Alu.mult,
                )
                nc.vector.scalar_tensor_tensor(
                    cwb[:], logits[:], m8[:, 1:2], w2[:, 0:1].to_broadcast([P, 8]),
                    op0=Alu.is_equal, op1=Alu.mult,
                )
                nc.vector.tensor_add(cw[:], cw[:], cwb[:])
                hw = hbuf.tile([P, ER], F16, tag="hw")
                # hw = (64*h) * 2^-6 * cw -> true h*cw scale; fp16 here so the
                # PE transpose is legal, cast to fp8 on the psum->sbuf copy
                nc.vector.scalar_tensor_tensor(
                    hw.rearrange("p (e r) -> p e r", e=E),
                    ps_h.rearrange("p (e r) -> p e r", e=E),
                    OUT_S,
                    cw[:, :, None].to_broadcast([P, E, R]),
                    op0=Alu.mult, op1=Alu.mult,
                )
                return hw

            def emit_router_pe(t, hw):
                """PE transposes of weighted h + copy back; fills pend[t] hwT."""
                ps_base, ps_h, _ = pend[t]
                ps_t = ps_t_pool.tile([P, ER], F16, tag="t")
                for j in range(4):
                    nc.tensor.transpose(
                        ps_t[:, ts(j, P)], hw[:, ts(j, P)], ident[:]
                    )
                hwT = hbuf.tile([P, 4, P], F8, tag="hwT")
                nc.vector.tensor_copy(hwT.rearrange("p a b -> p (a b)"), ps_t[:])
                pend[t] = (ps_base, ps_h, hwT)

            def emit_router(t):
                emit_router_pe(t, emit_router_dve(t))

            def emit_B_and_out(t):
                """LoRA phase B (fp8 DoubleRow) into base psum, bias, store."""
                ps_base, _, hwT = pend.pop(t)
                for j in range(2):
                    nc.tensor.matmul(
                        ps_base[:, 0:512], hwT[:, ts(j, 2), :],
                        bm_sb[:, ts(j, 2), 0:512],
                        start=False, stop=False,
                        perf_mode=DR, skip_group_check=True,
                    )
                    nc.tensor.matmul(
                        ps_base[:, 512:1024], hwT[:, ts(j, 2), :],
                        bm_sb[:, ts(j, 2), 512:1024],
                        start=False, stop=(j == 1),
                        perf_mode=DR, skip_group_check=True,
                    )
                out_sb = obuf.tile([P, D], F32, tag="out")
                nc.vector.scalar_tensor_tensor(
                    out_sb[:], ps_base[:], OUT_S, b2b_sb[:],
                    op0=Alu.mult, op1=Alu.add,
                )
                nc.scalar.dma_start(y_d[ts(t, P), :], out_sb[:])

            def alloc_psums(t):
                pend[t] = (
                    ps_base_pool.tile([P, D], F32, tag="base", name=f"base{t}"),
                    ps_h_pool.tile([P, ER], F32, tag="h", name=f"h{t}"),
                    None,
                )
                # The shared logits bank must never see start=True (a bank-wide
                # has_written clear would wipe the other tile's half). Instead
                # zero this tile's half; start=False matmuls then accumulate
                # onto 0 (bits set) or overwrite with v (bits clear) — both ok.
                nc.vector.memset(
                    ps_l_shared[:, (t % 2) * 32:(t % 2) * 32 + 32], 0.0
                )
                # ps_h takes only start=False matmuls (DoubleRow), zero it too
                ps_h = pend[t][1]
                nc.vector.memset(ps_h[:], 0.0)
                # on-device x8 cast for this tile (DVE, ~3us, runs in DVE
                # slack well before the tile's first LoRA matmul)
                x16_sb, x8_sb, _ = xts[t]
                nc.vector.tensor_copy(
                    x8_sb.rearrange("p a b -> p (a b)"),
                    x16_sb.rearrange("p a b -> p (a b)"),
                )

            # ---- startup: interleave phase A of tiles 0 and 1 so the PE has
            # two tiles of work while wcat groups stream in ----
            D_OFF = 4
            alloc_psums(0)
            alloc_psums(1)

            hw0 = None
            for g in range(NPAIR + D_OFF):
                if g < NPAIR:
                    emit_A_pair(0, g, late8=True)
                if g == NPAIR:
                    hw0 = emit_router_dve(0)
                gg = g - D_OFF
                if 0 <= gg < NPAIR:
                    emit_A_pair(1, gg, late8=True)
                if gg == NPAIR - 3:
                    # tile 0 transposes two pairs after its DVE chain began,
                    # so the PE doesn't stall on the chain
                    emit_router_pe(0, hw0)
                if gg == NPAIR - 2:
                    emit_B_and_out(0)
            hw_pend = {1: emit_router_dve(1)}

            # ---- steady state ----
            for t in range(2, TT - 1):
                if t >= 4:
                    issue_x(t, nc.gpsimd, nc.scalar)
                alloc_psums(t)
                for j in range(NPAIR):
                    emit_A_pair(t, j)
                    if j == 2 and (t - 1) in hw_pend:
                        # previous tile's transposes here: by pair 2 its DVE
                        # router chain (~2.5us incl ACT sigmoids) has had
                        # enough slack, so the PE doesn't stall on it
                        emit_router_pe(t - 1, hw_pend.pop(t - 1))
                    if j == 8:
                        # previous tile's phase B mid-A so its psum/base slot
                        # frees well before tile t+1 needs it
                        emit_B_and_out(t - 1)
                hw_pend[t] = emit_router_dve(t)

            # ---- last tile: router columns (L, h) stream first so the DVE
            # router chain overlaps the base-column streams; transposes are
            # injected mid-loop -> phase B follows the final matmul directly
            t = TT - 1
            issue_x(t, nc.gpsimd, nc.scalar)
            alloc_psums(t)
            x16_sb, x8_sb, xlo_sb = xts[t]
            ps_base, ps_h, _ = pend[t]
            ps_l = ps_l_shared[:, (t % 2) * 32:(t % 2) * 32 + 32]
            for j in range(NPAIR):
                for k in (2 * j, 2 * j + 1):
                    nc.tensor.matmul(
                        ps_l[:, 0:16], x16_sb[:, k, :], wc(k, D, NW),
                        start=False, stop=False, skip_group_check=True,
                    )
                if j == 4 and (t - 1) in hw_pend:
                    emit_router_pe(t - 1, hw_pend.pop(t - 1))
                if j >= NPAIR // 2:
                    # LoRA bunched late so the on-device x8 cast (DVE) for
                    # this tile completes before its first consumer
                    for jj in (j - NPAIR // 2, j):
                        nc.tensor.matmul(
                            ps_h[:, :], x8_sb[:, ts(jj, 2), :], a8p(jj),
                            start=False, stop=(jj == NPAIR - 1),
                            perf_mode=DR, skip_group_check=True,
                        )
                nc.tensor.matmul(
                    ps_l[:, 16:24], xlo_sb[:, ts(j, 2), :],
                    wg8_sb[:, ts(j, 2), :],
                    start=False, stop=(j == NPAIR - 1),
                    perf_mode=DR, skip_group_check=True,
                )
            hw_last = emit_router_dve(t)
            for k in range(KCH):
                st = k == 0
                nc.tensor.matmul(
                    ps_base[:, 0:512], x16_sb[:, k, :], wc(k, 0, 512),
                    start=st, stop=False, skip_group_check=True,
                )
                nc.tensor.matmul(
                    ps_base[:, 512:1024], x16_sb[:, k, :], wc(k, 512, 1024),
                    start=st, stop=(k == KCH - 1), skip_group_check=True,
                )
                if k == 4:
                    emit_B_and_out(t - 1)
                if k == 10:
                    emit_router_pe(t, hw_last)
            # drain: all 4 phase-B matmuls first (no DVE read interleaved —
            # a mid-stream psum read stalls the remaining matmuls on the
            # tile-granular WAR dependency), then a pipelined 4-way evac so
            # the output DMAs start as early as possible
            ps_base, _, hwT = pend.pop(TT - 1)
            for half in range(2):
                lo, hi = half * 512, half * 512 + 512
                for j in range(2):
                    nc.tensor.matmul(
                        ps_base[:, lo:hi], hwT[:, ts(j, 2), :],
                        bm_sb[:, ts(j, 2), lo:hi],
                        start=False, stop=(j == 1),
                        perf_mode=DR, skip_group_check=True,
                    )
            out_sb = obuf.tile([P, D], F32, tag="out")
            for q in range(4):
                lo, hi = q * 256, q * 256 + 256
                nc.vector.scalar_tensor_tensor(
                    out_sb[:, lo:hi], ps_base[:, lo:hi], OUT_S,
                    b2b_sb[:, lo:hi],
                    op0=Alu.mult, op1=Alu.add,
                )
                nc.scalar.dma_start(
                    y_d[ts(TT - 1, P), lo:hi], out_sb[:, lo:hi]
                )

    nc.finalize()
    return nc


def _prep_shared(Wg, W2, b2, A, Bm, scale):
    """Host-side weight layout prep (replicated across cores)."""
    f16, f32 = np.float16, np.float32
    f8 = ml_dtypes.float8_e4m3

    def chunked(a):
        # [H, N] -> [P, KCH, N]
        return np.ascontiguousarray(
            a.reshape(KCH, P, -1).transpose(1, 0, 2)
        )

    # wcat = [W2*64 | Wg_hi | Wg_lo] fp16
    wg_hi = Wg.astype(f16)
    wg_lo = (Wg.astype(f32) - wg_hi.astype(f32)).astype(f16)
    wcat = np.empty((H, NW), dtype=f16)
    wcat[:, 0:D] = (W2.astype(f32) * W_S).astype(f16)
    wcat[:, D:D + 8] = wg_hi
    wcat[:, D + 8:] = wg_lo
    wcat = chunked(wcat)

    a_flat = np.ascontiguousarray(A.transpose(1, 0, 2)).reshape(H, ER)
    a8 = chunked((a_flat.astype(f32) * W_S).astype(f8))
    wg8 = chunked((Wg.astype(f32) * W_S).astype(f8))

    # Bm with scale and 64x folded, [(e r), d] -> [128, 4, D] fp8
    bms = (Bm.astype(f32) * scale.astype(f32)[:, None, None]).reshape(ER, D)
    bms = np.ascontiguousarray(
        (bms * W_S).reshape(4, P, D).transpose(1, 0, 2)
    ).astype(f8)

    b2b = np.ascontiguousarray(
        np.broadcast_to(b2.astype(f32)[None, :], (P, D))
    )
    return wcat, a8, wg8, bms, b2b


def _prep_x_core(x_c):
    """Per-core x prep: fp16 hi + scaled-fp8 lo; [t, p, k, ti] layout.
    (x8 is derived on-device from x16.)"""
    f32 = np.float32
    f8 = ml_dtypes.float8_e4m3
    x16 = x_c.astype(np.float16)                            # [1024, 4096]
    xlo = ((x_c.astype(f32) - x16.astype(f32)) * XLO_S).astype(f8)

    def lay(a):
        return np.ascontiguousarray(
            a.reshape(TT, P, KCH, P).transpose(0, 3, 2, 1)
        )
    return lay(x16), lay(xlo)


def kernel(x, Wg, W2, b2, A, Bm, scale):
    x = np.asarray(x, dtype=np.float32)
    Wg = np.asarray(Wg, dtype=np.float32)
    W2 = np.asarray(W2, dtype=np.float32)
    b2 = np.asarray(b2, dtype=np.float32)
    A = np.asarray(A, dtype=np.float32)
    Bm = np.asarray(Bm, dtype=np.float32)
    scale = np.asarray(scale, dtype=np.float32)

    if "nc" not in _CACHE:
        _CACHE["nc"] = _build_nc()
    nc = _CACHE["nc"]

    wcat, a8, wg8, bms, b2b = _prep_shared(Wg, W2, b2, A, Bm, scale)
    in_maps = []
    for c in range(NCORES):
        x16, xlo = _prep_x_core(x[c])
        in_maps.append(
            {"x16": x16, "xlo": xlo, "wcat": wcat, "a8": a8,
             "wg8": wg8, "bm": bms, "b2b": b2b}
        )

    res = run_bass_kernel_spmd(nc, in_maps, core_ids=list(range(NCORES)))
    out = np.stack([res.results[c]["y"] for c in range(NCORES)], axis=0)
    return out.astype(np.float32)


# revision 46
# speedup vs baseline: 1.0484x; 1.0070x over previous
"""Trainium2 Bass kernel for DinoVisionTransformer Sparse-MoE FC2 (LoRA experts).

Computation (per token t):
    logits = x @ Wg                      -> top-2 softmax-renormalized weights
    out    = x @ W2 + b2 + sum_e cw[t,e] * scale[e] * (x @ A_e) @ B_e

Sharding: data-parallel over the batch dim (8 batch rows -> 8 NeuronCores,
1024 tokens each). All weights replicated.

Per-core kernel (fp16 base path, fp8e4m3 DoubleRow LoRA path, fp32 PSUM):
  All weight scales are folded by 64 (W2*64 fp16, A*64 fp8, Bm*scale*64 fp8)
  so PSUM accumulates 64*(base + delta); the final DVE pass multiplies by
  2^-6 and adds b2.
  Phase A (per pair of 128-deep k-chunks, contraction over H=4096):
    base: x16 stationary, W2 columns fp16 (2x512 per chunk)
    router hi: x16 @ [Wg_hi | Wg_lo] -> ps_l[0:16] (fp16, fp32 accum)
    LoRA: x8 pair stationary, A8 pair moving, fp8 DoubleRow (2 chunks/instr,
      2 cols/cycle) -> ps_h = 64*h.  x8 is cast on-device from x16 (DVE),
      saving 4.2 MB of HBM traffic per core.
    router lo: xlo8 (= (x - fp16(x))*4096 in fp8) @ wg8 (=Wg*64 fp8)
      DoubleRow -> ps_l[16:24] = 2^18 * correction
  Router (DVE): logits = reduce(ps_l[0:16]) + 2^-18*ps_l[16:24]; top-2 of 8
    via max8; w1 = sigmoid(l1-l2), w2 = 1-w1; dense cw by equality masks.
  hw = fp16(ps_h * cw * 2^-6) (true h*cw scale), PE-transposed (fp8
  transpose needs elem-step-2 output, so transpose fp16, cast to fp8 on
  the psum->sbuf copy); phase B: 4 fp8 DoubleRow matmuls accumulate
  64*delta into ps_base.
  Final: out = ps_base * 2^-6 + b2 (DVE scalar_tensor_tensor), DMA out.

DMA schedule: one transfer rides one HW-DGE queue set (~170 GB/s; the
~410 GB/s aggregate needs several in flight), and concurrent transfers
share bandwidth round-robin — so the critical startup streams (x16 t0/t1,
wcat groups) issue first on sync, while streams needed only from
mid-tile-0 onward are issued on gpsimd behind a gate-copy that reads the
x16 tiles (a real dependency that keeps them out of the early bandwidth
fight). Later x tiles prefetch through a 4-deep buffer pool.
"""

import sys

if "/opt/trn_rl_repo" not in sys.path:
    sys.path.insert(0, "/opt/trn_rl_repo")

import numpy as np
import ml_dtypes

import concourse.bass as bass  # noqa: F401  (registers types)
import concourse.mybir as mybir
import concourse.tile as tile
from concourse import bacc
from concourse.bass import ts
from concourse.bass_utils import run_bass_kernel_spmd
from concourse.masks import make_identity

P = 128
KCH = 32          # H / 128 contraction chunks
NPAIR = 16        # DoubleRow k-chunk pairs per tile
TT = 8            # 128-token tiles per core
H = 4096
D = 1024
E = 8
R = 64
ER = E * R        # 512
NW = D + 8 + 8    # 1040 fp16 wcat columns: [W2*64 | Wg_hi | Wg_lo]
NCORES = 8
XLO_S = 4096.0    # host scale on xlo before fp8 quantization
W_S = 64.0        # host scale on W2/A/Bm before quantization
CORR_S = 1.0 / (XLO_S * W_S)   # ps_l[16:24] -> logit units
OUT_S = 1.0 / W_S              # ps_base -> output units

F8 = mybir.dt.float8e4
F16 = mybir.dt.float16
F32 = mybir.dt.float32
DR = mybir.MatmulPerfMode.DoubleRow

_CACHE = {}


def _build_nc():
    nc = bacc.Bacc("TRN2")

    x16_d = nc.dram_tensor("x16", [TT, P, KCH, P], F16, kind="ExternalInput")
    xlo_d = nc.dram_tensor("xlo", [TT, P, KCH, P], F8, kind="ExternalInput")
    wcat_d = nc.dram_tensor("wcat", [P, KCH, NW], F16, kind="ExternalInput")
    a8_d = nc.dram_tensor("a8", [P, KCH, ER], F8, kind="ExternalInput")
    wg8_d = nc.dram_tensor("wg8", [P, KCH, 8], F8, kind="ExternalInput")
    bm_d = nc.dram_tensor("bm", [P, 4, D], F8, kind="ExternalInput")
    b2b_d = nc.dram_tensor("b2b", [P, D], F32, kind="ExternalInput")
    y_d = nc.dram_tensor("y", [TT * P, D], F32, kind="ExternalOutput")

    Sig = mybir.ActivationFunctionType.Sigmoid
    Alu = mybir.AluOpType

    with tile.TileContext(nc) as tc:
        with (
            tc.tile_pool(name="wres", bufs=1) as wres,
            tc.tile_pool(name="xin", bufs=4) as xin,
            tc.tile_pool(name="small", bufs=2) as small,
            tc.tile_pool(name="hbuf", bufs=2) as hbuf,
            tc.tile_pool(name="obuf", bufs=2) as obuf,
            tc.tile_pool(name="ps_base", bufs=2, space="PSUM") as ps_base_pool,
            tc.tile_pool(name="ps_h", bufs=2, space="PSUM") as ps_h_pool,
            tc.tile_pool(name="ps_l", bufs=1, space="PSUM") as ps_l_pool,
            tc.tile_pool(name="ps_t", bufs=1, space="PSUM") as ps_t_pool,
        ):
            # ---- startup DMA. The DMA engines share bandwidth round-robin
            # across outstanding transfers, so the critical-path streams
            # (x16 of tile 0, wcat groups) are issued FIRST from sync (the
            # empirically fast path), while everything needed only from
            # mid-tile-0 onward (xlo, a8 m1+, wg8, bm, b2b) is issued from
            # the vector engine AFTER the x8 cast of tile 0 — a real
            # dependency that keeps those transfers out of the early
            # bandwidth fight. ----
            xts = {}
            late_batches = {0: [], 1: [], 2: []}  # drained after cast(t)

            def issue_x(t0, x16_eng, xlo_eng):
                # x8 is not streamed: it is cast on-device from x16 (saves
                # 4.2 MB of HBM traffic). Cast is emitted in alloc_psums.
                x16_ = xin.tile([P, KCH, P], F16, tag="x16")
                x8_ = xin.tile([P, KCH, P], F8, tag="x8")
                xlo_ = xin.tile([P, KCH, P], F8, tag="xlo")
                if x16_eng is not None:
                    x16_eng.dma_start(x16_[:], x16_d[t0])
                if xlo_eng is not None:
                    xlo_eng.dma_start(xlo_[:], xlo_d[t0])
                xts[t0] = (x16_, x8_, xlo_)
                return x16_, xlo_

            ident = wres.tile([P, P], F16, tag="ident")
            make_identity(nc, ident[:])
            _, xlo0 = issue_x(0, nc.sync, None)
            wcat_sb = []
            a8_sb = []
            wcat0_ = wres.tile([P, 4, NW], F16, tag="wcat0", name="wcat0")
            wcat_sb.append(wcat0_)
            nc.sync.dma_start(wcat0_[:], wcat_d[:, ts(0, 4), :])
            _, xlo1 = issue_x(1, nc.sync, None)
            for g in range(1, 8):
                t_ = wres.tile([P, 4, NW], F16, tag=f"wcat{g}")
                nc.sync.dma_start(t_[:], wcat_d[:, ts(g, 4), :])
                wcat_sb.append(t_)
                if g == 3:
                    a_ = wres.tile([P, 8, ER], F8, tag="a80")
                    nc.sync.dma_start(a_[:], a8_d[:, ts(0, 8), :])
                    a8_sb.append(a_)
            wg8_sb = wres.tile([P, KCH, 8], F8, tag="wg8")
            bm_sb = wres.tile([P, 4, D], F8, tag="bm")
            b2b_sb = wres.tile([P, D], F32, tag="b2b")
            for m in range(1, 4):
                a_ = wres.tile([P, 8, ER], F8, tag=f"a8{m}", name=f"a8{m}")
                a8_sb.append(a_)
            x16_2, xlo_2 = issue_x(2, None, None)
            x16_3, xlo_3 = issue_x(3, None, None)
            # deferred issues, in consumption order, drained on gpsimd after
            # each early cast — so their transfers start only once the
            # critical tile-0/1 x16 + wcat stream has landed
            late_batches[0] = [
                (xlo0[:], xlo_d[0]),
                (wg8_sb[:], wg8_d[:]),
                (a8_sb[1][:], a8_d[:, ts(1, 8), :]),
                (a8_sb[2][:], a8_d[:, ts(2, 8), :]),
                (a8_sb[3][:], a8_d[:, ts(3, 8), :]),
            ]
            late_batches[1] = [
                (xlo1[:], xlo_d[1]),
                (bm_sb[:], bm_d[:]),
                (b2b_sb[:], b2b_d[:]),
                (x16_2[:], x16_d[2]),
                (xlo_2[:], xlo_d[2]),
            ]
            late_batches[2] = [
                (x16_3[:], x16_d[3]),
                (xlo_3[:], xlo_d[3]),
            ]
            # drain the deferred batches on gpsimd, each gated behind a tiny
            # copy that reads the corresponding x16 tile — so these
            # transfers start only after the critical early streams landed.
            # Batch 2 needs no gate: its destinations reuse tile-0 xin
            # buffers, so the WAR dependency throttles them naturally.
            for gi in (0, 1):
                gate_ = small.tile([P, 8], F16, tag="gate", name=f"gate{gi}")
                nc.gpsimd.tensor_copy(gate_[:], xts[gi][0][:, 0, 0:8])
                for dst, src in late_batches.pop(gi):
                    nc.gpsimd.dma_start(dst, src)
            for dst, src in late_batches.pop(2):
                nc.gpsimd.dma_start(dst, src)

            def wc(k, lo, hi):
                return wcat_sb[k // 4][:, k % 4, lo:hi]

            def a8p(j):
                return a8_sb[j // 4][:, (j % 4) * 2:(j % 4) * 2 + 2, :]

            # shared logits psum bank: tile t uses half (t % 2).
            # cols [0:16] = x16 @ [Wg_hi | Wg_lo]; cols [16:24] = 2^18 x the
            # xlo correction (fp8 DoubleRow; rescaled on the DVE afterwards)
            ps_l_shared = ps_l_pool.tile([P, 64], F32, tag="l")

            pend = {}   # t -> (ps_base, ps_h, hwT or None)

            def emit_A_pair(t, j, late8=False, warm_only=False):
                """Phase-A matmuls for k-chunk pair j (chunks 2j, 2j+1).

                late8: bunch the fp8 LoRA + xlo-correction DoubleRow matmuls
                into the second half of the pair loop (two per slot) so the
                fp8 x streams can be issued after the first wcat groups."""
                x16_sb, x8_sb, xlo_sb = xts[t]
                ps_base, ps_h, _ = pend[t]
                ps_l = ps_l_shared[:, (t % 2) * 32:(t % 2) * 32 + 32]

                def lora(jj):
                    nc.tensor.matmul(
                        ps_h[:, :], x8_sb[:, ts(jj, 2), :], a8p(jj),
                        start=False, stop=(jj == NPAIR - 1),
                        perf_mode=DR, skip_group_check=True,
                    )

                def xcorr(jj):
                    nc.tensor.matmul(
                        ps_l[:, 16:24], xlo_sb[:, ts(jj, 2), :],
                        wg8_sb[:, ts(jj, 2), :],
                        start=False, stop=(jj == NPAIR - 1),
                        perf_mode=DR, skip_group_check=True,
                    )

                for k in (2 * j, 2 * j + 1):
                    st = k == 0
                    # order: tiny-N matmuls sit between 512-col streams so
                    # their self-loading weight fetches hide under the streams
                    nc.tensor.matmul(
                        ps_base[:, 0:512], x16_sb[:, k, :], wc(k, 0, 512),
                        start=st, stop=False, skip_group_check=True,
                    )
                    if not warm_only:
                        nc.tensor.matmul(
                            ps_l[:, 0:16], x16_sb[:, k, :], wc(k, D, NW),
                            start=False, stop=False, skip_group_check=True,
                        )
                    nc.tensor.matmul(
                        ps_base[:, 512:1024], x16_sb[:, k, :], wc(k, 512, 1024),
                        start=st, stop=False, skip_group_check=True,
                    )
                    if warm_only:
                        continue
                    if k % 2 == 1:
                        if late8:
                            if j >= NPAIR // 2:
                                for jj in (j - NPAIR // 2, j):
                                    lora(jj)
                                    xcorr(jj)
                        else:
                            lora(j)
                            xcorr(j)

            def emit_router_dve(t):
                """Router math + h-weighting (DVE/ACT only); returns hw8."""
                ps_base, ps_h, _ = pend[t]
                ps_l = ps_l_shared[:, (t % 2) * 32:(t % 2) * 32 + 32]
                logits = small.tile([P, 8], F32, tag="logits")
                nc.vector.tensor_reduce(
                    logits[:],
                    ps_l[:, 0:16].rearrange("p (s j) -> p j s", s=2),
                    axis=mybir.AxisListType.X,
                    op=Alu.add,
                )
                nc.vector.scalar_tensor_tensor(
                    logits[:], ps_l[:, 16:24], CORR_S, logits[:],
                    op0=Alu.mult, op1=Alu.add,
                )
                m8 = small.tile([P, 8], F32, tag="m8")
                nc.vector.max(m8[:], logits[:])
                g_ = small.tile([P, 1], F32, tag="gap")
                nc.vector.tensor_sub(g_[:], m8[:, 0:1], m8[:, 1:2])
                w1 = small.tile([P, 1], F32, tag="w1")
                nc.scalar.activation(w1[:], g_[:], Sig)
                w2 = small.tile([P, 1], F32, tag="w2")
                nc.scalar.activation(w2[:], g_[:], Sig, scale=-1.0)
                cw = small.tile([P, 8], F32, tag="cw")
                cwb = small.tile([P, 8], F32, tag="cwb")
                nc.vector.scalar_tensor_tensor(
                    cw[:], logits[:], m8[:, 0:1], w1[:, 0:1].to_broadcast([P, 8]),
                    op0=Alu.is_equal, op1=Alu.mult,
                )
                nc.vector.scalar_tensor_tensor(
                    cwb[:], logits[:], m8[:, 1:2], w2[:, 0:1].to_broadcast([P, 8]),
                    op0=Alu.is_equal, op1=Alu.mult,
                )
                nc.vector.tensor_add(cw[:], cw[:], cwb[:])
                hw = hbuf.tile([P, ER], F16, tag="hw")
                # hw = (64*h) * 2^-6 * cw -> true h*cw scale; fp16 here so the
                # PE transpose is legal, cast to fp8 on the psum->sbuf copy
                nc.vector.scalar_tensor_tensor(
                    hw.rearrange("p (e r) -> p e r", e=E),
                    ps_h.rearrange("p (e r) -> p e r", e=E),
                    OUT_S,
                    cw[:, :, None].to_broadcast([P, E, R]),
                    op0=Alu.mult, op1=Alu.mult,
                )
                return hw

            def emit_router_pe(t, hw):
                """PE transposes of weighted h + copy back; fills pend[t] hwT."""
                ps_base, ps_h, _ = pend[t]
                ps_t = ps_t_pool.tile([P, ER], F16, tag="t")
                for j in range(4):
                    nc.tensor.transpose(
                        ps_t[:, ts(j, P)], hw[:, ts(j, P)], ident[:]
                    )
                hwT = hbuf.tile([P, 4, P], F8, tag="hwT")
                nc.vector.tensor_copy(hwT.rearrange("p a b -> p (a b)"), ps_t[:])
                pend[t] = (ps_base, ps_h, hwT)

            def emit_router(t):
                emit_router_pe(t, emit_router_dve(t))

            def emit_B_and_out(t):
                """LoRA phase B (fp8 DoubleRow) into base psum, bias, store."""
                ps_base, _, hwT = pend.pop(t)
                for j in range(2):
                    nc.tensor.matmul(
                        ps_base[:, 0:512], hwT[:, ts(j, 2), :],
                        bm_sb[:, ts(j, 2), 0:512],
                        start=False, stop=False,
                        perf_mode=DR, skip_group_check=True,
                    )
                    nc.tensor.matmul(
                        ps_base[:, 512:1024], hwT[:, ts(j, 2), :],
                        bm_sb[:, ts(j, 2), 512:1024],
                        start=False, stop=(j == 1),
                        perf_mode=DR, skip_group_check=True,
                    )
                out_sb = obuf.tile([P, D], F32, tag="out")
                nc.vector.scalar_tensor_tensor(
                    out_sb[:], ps_base[:], OUT_S, b2b_sb[:],
                    op0=Alu.mult, op1=Alu.add,
                )
                nc.scalar.dma_start(y_d[ts(t, P), :], out_sb[:])

            def alloc_psums(t, do_cast=True):
                pend[t] = (
                    ps_base_pool.tile([P, D], F32, tag="base", name=f"base{t}"),
                    ps_h_pool.tile([P, ER], F32, tag="h", name=f"h{t}"),
                    None,
                )
                # The shared logits bank must never see start=True (a bank-wide
                # has_written clear would wipe the other tile's half). Instead
                # zero this tile's half; start=False matmuls then accumulate
                # onto 0 (bits set) or overwrite with v (bits clear) — both ok.
                nc.vector.memset(
                    ps_l_shared[:, (t % 2) * 32:(t % 2) * 32 + 32], 0.0
                )
                # ps_h takes only start=False matmuls (DoubleRow), zero it too
                ps_h = pend[t][1]
                nc.vector.memset(ps_h[:], 0.0)
                # on-device x8 cast for this tile (DVE, ~3us, runs in DVE
                # slack well before the tile's first LoRA matmul)
                if do_cast:
                    emit_cast(t)

            def emit_cast(t):
                x16_sb, x8_sb, _ = xts[t]
                nc.vector.tensor_copy(
                    x8_sb.rearrange("p a b -> p (a b)"),
                    x16_sb.rearrange("p a b -> p (a b)"),
                )

            # ---- startup: interleave phase A of tiles 0 and 1 so the PE has
            # two tiles of work while wcat groups stream in ----
            D_OFF = 4
            alloc_psums(0)
            alloc_psums(1)

            hw0 = None
            for g in range(NPAIR + D_OFF):
                if g < NPAIR:
                    emit_A_pair(0, g, late8=True)
                if g == NPAIR:
                    hw0 = emit_router_dve(0)
                gg = g - D_OFF
                if 0 <= gg < NPAIR:
                    emit_A_pair(1, gg, late8=True)
                if gg == NPAIR - 3:
                    # tile 0 transposes two pairs after its DVE chain began,
                    # so the PE doesn't stall on the chain
                    emit_router_pe(0, hw0)
                if gg == NPAIR - 2:
                    emit_B_and_out(0)
            hw_pend = {1: emit_router_dve(1)}

            # ---- steady state ----
            for t in range(2, TT - 1):
                if t >= 4:
                    issue_x(t, nc.gpsimd, nc.scalar)
                alloc_psums(t)
                if t == 6:
                    # issue + cast tile 7 early so the last tile's loop1 can
                    # interleave its LoRA matmuls per-pair without stalling
                    issue_x(7, nc.gpsimd, nc.scalar)
                    emit_cast(7)
                for j in range(NPAIR):
                    emit_A_pair(t, j)
                    if j == 2 and (t - 1) in hw_pend:
                        # previous tile's transposes here: by pair 2 its DVE
                        # router chain (~2.5us incl ACT sigmoids) has had
                        # enough slack, so the PE doesn't stall on it
                        emit_router_pe(t - 1, hw_pend.pop(t - 1))
                    if j == 8:
                        # previous tile's phase B mid-A so its psum/base slot
                        # frees well before tile t+1 needs it
                        emit_B_and_out(t - 1)
                hw_pend[t] = emit_router_dve(t)

            # ---- last tile: router columns (L, h) stream first so the DVE
            # router chain overlaps the base-column streams; transposes are
            # injected mid-loop -> phase B follows the final matmul directly
            t = TT - 1
            alloc_psums(t, do_cast=False)
            x16_sb, x8_sb, xlo_sb = xts[t]
            ps_base, ps_h, _ = pend[t]
            ps_l = ps_l_shared[:, (t % 2) * 32:(t % 2) * 32 + 32]
            for j in range(NPAIR):
                # LoRA per-pair: its 213ns stream hides the tiny router
                # matmuls' weight loads (x8 was cast back at tile 6)
                nc.tensor.matmul(
                    ps_h[:, :], x8_sb[:, ts(j, 2), :], a8p(j),
                    start=False, stop=(j == NPAIR - 1),
                    perf_mode=DR, skip_group_check=True,
                )
                for k in (2 * j, 2 * j + 1):
                    nc.tensor.matmul(
                        ps_l[:, 0:16], x16_sb[:, k, :], wc(k, D, NW),
                        start=False, stop=False, skip_group_check=True,
                    )
                if j == 4 and (t - 1) in hw_pend:
                    emit_router_pe(t - 1, hw_pend.pop(t - 1))
                nc.tensor.matmul(
                    ps_l[:, 16:24], xlo_sb[:, ts(j, 2), :],
                    wg8_sb[:, ts(j, 2), :],
                    start=False, stop=(j == NPAIR - 1),
                    perf_mode=DR, skip_group_check=True,
                )
            hw_last = emit_router_dve(t)
            for k in range(KCH):
                st = k == 0
                nc.tensor.matmul(
                    ps_base[:, 0:512], x16_sb[:, k, :], wc(k, 0, 512),
                    start=st, stop=False, skip_group_check=True,
                )
                nc.tensor.matmul(
                    ps_base[:, 512:1024], x16_sb[:, k, :], wc(k, 512, 1024),
                    start=st, stop=(k == KCH - 1), skip_group_check=True,
                )
                if k == 4:
                    emit_B_and_out(t - 1)
                if k == 10:
                    emit_router_pe(t, hw_last)
            # drain: all 4 phase-B matmuls first (no DVE read interleaved —
            # a mid-stream psum read stalls the remaining matmuls on the
            # tile-granular WAR dependency), then a pipelined 4-way evac so
            # the output DMAs start as early as possible
            ps_base, _, hwT = pend.pop(TT - 1)
            for half in range(2):
                lo, hi = half * 512, half * 512 + 512
                for j in range(2):
                    nc.tensor.matmul(
                        ps_base[:, lo:hi], hwT[:, ts(j, 2), :],
                        bm_sb[:, ts(j, 2), lo:hi],
                        start=False, stop=(j == 1),
                        perf_mode=DR, skip_group_check=True,
                    )
            out_sb = obuf.tile([P, D], F32, tag="out")
            for q in range(4):
                lo, hi = q * 256, q * 256 + 256
                nc.vector.scalar_tensor_tensor(
                    out_sb[:, lo:hi], ps_base[:, lo:hi], OUT_S,
                    b2b_sb[:, lo:hi],
                    op0=Alu.mult, op1=Alu.add,
                )
                # alternate issue engines: serial dma_start issues (~0.6us
                # each) on one engine would add directly to the kernel tail
                eng = nc.scalar if q % 2 == 0 else nc.sync
                eng.dma_start(
                    y_d[ts(TT - 1, P), lo:hi], out_sb[:, lo:hi]
                )

    nc.finalize()
    return nc


def _prep_shared(Wg, W2, b2, A, Bm, scale):
    """Host-side weight layout prep (replicated across cores)."""
    f16, f32 = np.float16, np.float32
    f8 = ml_dtypes.float8_e4m3

    def chunked(a):
        # [H, N] -> [P, KCH, N]
        return np.ascontiguousarray(
            a.reshape(KCH, P, -1).transpose(1, 0, 2)
        )

    # wcat = [W2*64 | Wg_hi | Wg_lo] fp16
    wg_hi = Wg.astype(f16)
    wg_lo = (Wg.astype(f32) - wg_hi.astype(f32)).astype(f16)
    wcat = np.empty((H, NW), dtype=f16)
    wcat[:, 0:D] = (W2.astype(f32) * W_S).astype(f16)
    wcat[:, D:D + 8] = wg_hi
    wcat[:, D + 8:] = wg_lo
    wcat = chunked(wcat)

    a_flat = np.ascontiguousarray(A.transpose(1, 0, 2)).reshape(H, ER)
    a8 = chunked((a_flat.astype(f32) * W_S).astype(f8))
    wg8 = chunked((Wg.astype(f32) * W_S).astype(f8))

    # Bm with scale and 64x folded, [(e r), d] -> [128, 4, D] fp8
    bms = (Bm.astype(f32) * scale.astype(f32)[:, None, None]).reshape(ER, D)
    bms = np.ascontiguousarray(
        (bms * W_S).reshape(4, P, D).transpose(1, 0, 2)
    ).astype(f8)

    b2b = np.ascontiguousarray(
        np.broadcast_to(b2.astype(f32)[None, :], (P, D))
    )
    return wcat, a8, wg8, bms, b2b


def _prep_x_core(x_c):
    """Per-core x prep: fp16 hi + scaled-fp8 lo; [t, p, k, ti] layout.
    (x8 is derived on-device from x16.)"""
    f32 = np.float32
    f8 = ml_dtypes.float8_e4m3
    x16 = x_c.astype(np.float16)                            # [1024, 4096]
    xlo = ((x_c.astype(f32) - x16.astype(f32)) * XLO_S).astype(f8)

    def lay(a):
        return np.ascontiguousarray(
            a.reshape(TT, P, KCH, P).transpose(0, 3, 2, 1)
        )
    return lay(x16), lay(xlo)


def kernel(x, Wg, W2, b2, A, Bm, scale):
    x = np.asarray(x, dtype=np.float32)
    Wg = np.asarray(Wg, dtype=np.float32)
    W2 = np.asarray(W2, dtype=np.float32)
    b2 = np.asarray(b2, dtype=np.float32)
    A = np.asarray(A, dtype=np.float32)
    Bm = np.asarray(Bm, dtype=np.float32)
    scale = np.asarray(scale, dtype=np.float32)

    if "nc" not in _CACHE:
        _CACHE["nc"] = _build_nc()
    nc = _CACHE["nc"]

    wcat, a8, wg8, bms, b2b = _prep_shared(Wg, W2, b2, A, Bm, scale)
    in_maps = []
    for c in range(NCORES):
        x16, xlo = _prep_x_core(x[c])
        in_maps.append(
            {"x16": x16, "xlo": xlo, "wcat": wcat, "a8": a8,
             "wg8": wg8, "bm": bms, "b2b": b2b}
        )

    res = run_bass_kernel_spmd(nc, in_maps, core_ids=list(range(NCORES)))
    out = np.stack([res.results[c]["y"] for c in range(NCORES)], axis=0)
    return out.astype(np.float32)
